# revision 22
# baseline (speedup 1.0000x reference)
"""Bayesian uncertainty distance kernel for TRN2 (8 NeuronCores, SPMD).

Math (per reference):
    W_s  = weight_mu + eps_w[s] * softplus(weight_rho)          [S,D,D]
    b_s  = bias_mu   + eps_b[s] * softplus(bias_rho)            [S,D]
    qt_s = query @ W_s + b_s                                    [S,Q,D]
    d2_s = ||qt_s||^2 - 2 qt_s.proto^T + ||proto||^2            [S,Q,P]
    mean = mean_s sqrt(d2_s);  std = std_s(sqrt(d2_s), ddof=1)

Sharding: data-parallel over Q (8192 -> 8 x 1024). Everything else replicated.

Design (v2, fp8 DoubleRow):
  - x_s := -2*qt_s - 2*b_s is split into fp8(e4m3) hi+lo pieces
    (xh = e4m3(fp16(x)), xl = e4m3(x - xh)) stored interleaved so the PE
    contracts x~ = xh+xl against fp8 prototypes y~ with DoubleRow matmuls
    (0.5 cyc/col): cross needs 2 DR MMs per 512-col chunk instead of 2
    fp16 passes -> 2x PE.  y~'s rounding is constant across samples so it
    cancels exactly in the sample std; x~'s hi+lo error is ~fp13.
  - ||qt||^2 enters via the ACT Sqrt's per-partition bias (qn columns),
    built by an all-ones select matmul over x2=fp16(x^2) plus a tiny PE
    transpose; pn enters as a rank-1 fp8 DR seed (pn split hi+lo with a
    [2;1]-valued stationary row pair, since e4m3 max is 240 < max pn).
  - Variance via the closed form sum_s d2 = qnsum + 10*pn + xsum.y~ with
    all matmuls in float32r (1 cyc/col >=256 cols, numerically exact
    fp32): xsum accumulated on-PE with fp8 identity-pair DR matmuls so it
    matches sum_s (xh_s+xl_s) bit-exactly -> first-order rounding cancels
    in u = ss - macc^2/S.  qnsum rides the final Sqrt's bias (qnsum/9).
  - macc = sum_s dist: split across engines (DVE adds, Pool adds, and PE
    float32r identity matmuls into PSUM) to balance the four queues; the
    per-sample sqrt stays on ACT (its table op) with dist written once.

The host does only O(S*D^2) prep in numpy (softplus, W_s, transposes, pn).
"""

import os
import numpy as np
import ml_dtypes

import concourse.bass as bass
import concourse.mybir as mybir
import concourse.tile as tile
from concourse import bacc, bass_utils

AF = mybir.ActivationFunctionType
ALU = mybir.AluOpType
DR = mybir.MatmulPerfMode.DoubleRow

F32 = mybir.dt.float32
F32R = mybir.dt.float32r
F16 = mybir.dt.float16
F8 = mybir.dt.float8e4
E4 = ml_dtypes.float8_e4m3

NCORES = 8
D = 256
Q_FULL = 8192
P = 2048
S = 10
QLOC = Q_FULL // NCORES  # 1024
ET = D // 128  # 2 e-tiles
DT = D // 128  # 2 d-tiles
QT = QLOC // 128  # 8 q-tiles per core
EQ = ET * QLOC  # 2048: one hi- or lo-plane of x

# tuning knobs: sqrt samples routed to Pool (via add+pow); DVE/Pool column
# split point for the macc accumulation adds
SQRT_POOL = ()  # gpsimd cannot read PSUM, so sqrt stays on ACT
DSPL = 1216

_CACHE = {}
LAST_RESULTS = None


def _build_bass(num_devices=NCORES, dbg=False):
    nc = bacc.Bacc(
        "TRN2",
        target_bir_lowering=False,
        debug=False,
        num_devices=num_devices,
    )
    ins = {}
    ins["qT32"] = nc.dram_tensor("qT32", [128, DT * QLOC], F32R, kind="ExternalInput").ap()
    ins["W32"] = nc.dram_tensor("W32", [S, 128, DT * 256], F32R, kind="ExternalInput").ap()
    ins["b2T"] = nc.dram_tensor("b2T", [128, ET * S], F32, kind="ExternalInput").ap()
    ins["y8dup"] = nc.dram_tensor("y8dup", [128, 2 * ET * P], F8, kind="ExternalInput").ap()
    ins["yT16"] = nc.dram_tensor("yT16", [128, ET * P], F16, kind="ExternalInput").ap()
    ins["seedw8"] = nc.dram_tensor("seedw8", [1, 256], F8, kind="ExternalInput").ap()
    ins["seedy8"] = nc.dram_tensor("seedy8", [1, 2 * P], F8, kind="ExternalInput").ap()
    ins["pn10_16"] = nc.dram_tensor("pn10_16", [1, P], F16, kind="ExternalInput").ap()
    ins["ones16"] = nc.dram_tensor("ones16", [1, 128], F16, kind="ExternalInput").ap()
    ins["eye8dup"] = nc.dram_tensor("eye8dup", [128, 256], F8, kind="ExternalInput").ap()
    ins["eyeT10"] = nc.dram_tensor("eyeT10", [10, 10], F32, kind="ExternalInput").ap()
    ins["sel16"] = nc.dram_tensor("sel16", [128, S * 16], F16, kind="ExternalInput").ap()
    mean_o = nc.dram_tensor("mean_o", [QLOC, P], F32, kind="ExternalOutput").ap()
    std_o = nc.dram_tensor("std_o", [QLOC, P], F32, kind="ExternalOutput").ap()
    dbg_o = None
    if dbg:
        dbg_o = {
            "dbg_xhl": nc.dram_tensor("dbg_xhl", [S, 128, 2 * EQ], F8, kind="ExternalOutput").ap(),
            "dbg_qncol": nc.dram_tensor("dbg_qncol", [128, QT * S], F32, kind="ExternalOutput").ap(),
            "dbg_xsum": nc.dram_tensor("dbg_xsum", [128, EQ], F16, kind="ExternalOutput").ap(),
        }

    with tile.TileContext(nc) as tc:
        _kernel_body(tc, ins, mean_o, std_o, dbg_o)
    nc.compile()
    return nc


def _kernel_body(tc, ins, mean_o, std_o, dbg_o=None):
    nc = tc.nc
    from contextlib import ExitStack

    ctx = ExitStack()
    with ctx:
        cpool = ctx.enter_context(tc.tile_pool(name="consts", bufs=1))
        wpool = ctx.enter_context(tc.tile_pool(name="wpool", bufs=2))
        xmpool = ctx.enter_context(tc.tile_pool(name="xmpool", bufs=3))
        x2pool = ctx.enter_context(tc.tile_pool(name="x2pool", bufs=2))
        xhlpool = ctx.enter_context(tc.tile_pool(name="xhlpool", bufs=S))
        qnpool = ctx.enter_context(tc.tile_pool(name="qnpool", bufs=1))
        xsumpool = ctx.enter_context(tc.tile_pool(name="xsumpool", bufs=1))
        distpool = ctx.enter_context(tc.tile_pool(name="distpool", bufs=2))
        maccpool = ctx.enter_context(tc.tile_pool(name="maccpool", bufs=2))
        finpool = ctx.enter_context(tc.tile_pool(name="finpool", bufs=2))
        outpool = ctx.enter_context(tc.tile_pool(name="outpool", bufs=2))
        pp = ctx.enter_context(tc.tile_pool(name="pp", bufs=2, space="PSUM"))

        # ---- constants into SBUF ----
        qT_t = cpool.tile([128, DT * QLOC], F32R)
        nc.sync.dma_start(qT_t[:], ins["qT32"])
        b2_t = cpool.tile([128, ET * S], F32)
        nc.sync.dma_start(b2_t[:], ins["b2T"])
        y8_t = cpool.tile([128, 2 * ET * P], F8)
        nc.sync.dma_start(y8_t[:], ins["y8dup"])
        y16_t = cpool.tile([128, ET * P], F16)
        nc.sync.dma_start(y16_t[:], ins["yT16"])
        seedw_t = cpool.tile([1, 256], F8)
        nc.sync.dma_start(seedw_t[:], ins["seedw8"])
        seedy_t = cpool.tile([1, 2 * P], F8)
        nc.sync.dma_start(seedy_t[:], ins["seedy8"])
        pn10_t = cpool.tile([1, P], F16)
        nc.sync.dma_start(pn10_t[:], ins["pn10_16"])
        ones16_t = cpool.tile([1, 128], F16)
        nc.sync.dma_start(ones16_t[:], ins["ones16"])
        eye8d_t = cpool.tile([128, 256], F8)
        nc.sync.dma_start(eye8d_t[:], ins["eye8dup"])
        eyeT10_t = cpool.tile([10, 10], F32)
        nc.sync.dma_start(eyeT10_t[:], ins["eyeT10"])
        sel_t = cpool.tile([128, S * 16], F16)
        nc.sync.dma_start(sel_t[:], ins["sel16"])

        # qn machinery outputs
        qn_sq = qnpool.tile([10, QLOC], F32)  # qn rows, one partition per s
        qncol = qnpool.tile([128, QT * S], F32)  # qn columns per (qt, s)
        qn9 = qnpool.tile([128, QT], F32)  # qnsum/9 per qt (std bias)
        xsum16 = xsumpool.tile([128, EQ], F16)  # fp16 xsum for the exact ss MMs

        # 3D views for DoubleRow k-tile pairs
        seedw3 = seedw_t[:].rearrange("p (two m) -> p two m", two=2)
        seedy3 = seedy_t[:].rearrange("p (two x) -> p two x", two=2)
        eye8d3 = eye8d_t[:].rearrange("p (two m) -> p two m", two=2)
        y8d3 = y8_t[:].rearrange("p (two x) -> p two x", two=2)

        x_tiles = []
        # ---------- phase 1: per-sample transformed queries ----------
        qnp = None
        for s in range(S):
            w_t = wpool.tile([128, DT * 256], F32R, tag="w", name=f"w{s}")
            nc.sync.dma_start(w_t[:], ins["W32"][s])
            xhl_t = xhlpool.tile([128, 2 * EQ], F8, tag="x", name=f"x{s}")
            x_tiles.append(xhl_t)
            x2_t = x2pool.tile([128, EQ], F16, tag="x2", name=f"x2_{s}")
            for et in range(ET):
                for qc in range(2):
                    qp = pp.tile([128, 512], F32, tag="d2", name=f"qp{s}_{et}_{qc}")
                    for dt_ in range(DT):
                        nc.tensor.matmul(
                            qp[:],
                            lhsT=w_t[:, dt_ * 256 + et * 128 : dt_ * 256 + et * 128 + 128],
                            rhs=qT_t[:, dt_ * QLOC + qc * 512 : dt_ * QLOC + qc * 512 + 512],
                            start=(dt_ == 0),
                            stop=(dt_ == DT - 1),
                        )
                    o = et * QLOC + qc * 512
                    xm = xmpool.tile([128, 512], F16, tag="xm", name=f"xm{s}_{et}_{qc}")
                    # x = -2*qt - 2*b (fp16) on ACT (Identity with bias AP)
                    nc.scalar.activation(
                        xm[:], qp[:], AF.Identity,
                        bias=b2_t[:, et * S + s : et * S + s + 1],
                        scale=-2.0,
                    )
                    # hi piece: fp8 round of x (DVE dtype-convert copy)
                    nc.vector.tensor_copy(xhl_t[:, o : o + 512], xm[:])
                    # lo piece: fp8(x - xh) on Pool
                    nc.gpsimd.tensor_sub(
                        xhl_t[:, EQ + o : EQ + o + 512], xm[:], xhl_t[:, o : o + 512]
                    )
                    # x2 = x^2 fp16 on DVE (2-byte fast path)
                    nc.vector.tensor_mul(x2_t[:, o : o + 512], xm[:], xm[:])
            # qn rows: select-matmul accumulates 0.25*sum_e x2 into psum row s
            if s == 0:
                qnp = pp.tile([10, QLOC], F32, tag="acc", bufs=1, name="qnp")
            for qc in range(2):
                for et in range(ET):
                    nc.tensor.matmul(
                        qnp[:, qc * 512 : qc * 512 + 512],
                        lhsT=sel_t[:, s * 16 : s * 16 + 10],
                        rhs=x2_t[:, et * QLOC + qc * 512 : et * QLOC + qc * 512 + 512],
                        start=(s == 0 and et == 0),
                        stop=(s == S - 1 and et == ET - 1),
                        skip_group_check=True,
                    )

        # qn rows -> sbuf (scale 0.25 compensates x=(2qt+2b): qn=||x/2||^2)
        nc.scalar.activation(qn_sq[:, :], qnp[:, :], AF.Copy, scale=0.25)
        # transpose 128-blocks to get per-(qt,s) bias columns
        for qt_ in range(QT):
            qtp = pp.tile([128, 10], F32, tag="d2", name=f"qtp{qt_}")
            nc.tensor.matmul(
                qtp[:],
                lhsT=qn_sq[0:10, qt_ * 128 : qt_ * 128 + 128],
                rhs=eyeT10_t[:],
                is_transpose=True,
            )
            nc.scalar.copy(qncol[:, qt_ * S : qt_ * S + S], qtp[:])
            nc.vector.tensor_reduce(
                qn9[:, qt_ : qt_ + 1],
                qncol[:, qt_ * S : qt_ * S + S],
                axis=mybir.AxisListType.X,
                op=ALU.add,
            )
        nc.vector.tensor_scalar_mul(qn9[:], qn9[:], 1.0 / (S - 1))

        # xsum = sum_s (xh_s + xl_s) via fp8 identity-pair DR matmuls
        xsp = pp.tile([128, EQ], F32, tag="acc", bufs=1, name="xsp")
        for s in range(S):
            x3 = x_tiles[s][:].rearrange("p (two x) -> p two x", two=2)
            for et in range(ET):
                for qc in range(2):
                    o = et * QLOC + qc * 512
                    nc.tensor.matmul(
                        xsp[:, o : o + 512],
                        lhsT=eye8d3,
                        rhs=x3[:, :, o : o + 512],
                        start=(s == 0),
                        stop=(s == S - 1),
                        perf_mode=DR,
                        skip_group_check=True,
                    )
        nc.vector.tensor_copy(xsum16[:, 0:QLOC], xsp[:, 0:QLOC])
        nc.scalar.copy(xsum16[:, QLOC:EQ], xsp[:, QLOC:EQ])

        if dbg_o is not None:
            for s in range(S):
                nc.sync.dma_start(dbg_o["dbg_xhl"][s], x_tiles[s][:])
            nc.sync.dma_start(dbg_o["dbg_qncol"], qncol[:])
            nc.sync.dma_start(dbg_o["dbg_xsum"], xsum16[:])

        # ---------- phase 2: distances, moments, outputs ----------
        # Tail work of qt_ is deferred into qt_+1's s-loop (stage A at s==1,
        # stage B at s==3) so the PE queue never stalls on not-yet-emitted
        # ACT/DVE work.
        pend_tail_a = []
        pend_tail_b = []
        for qt_ in range(QT):
            maccD = maccpool.tile([128, P], F32, tag="maccD", name=f"maccD{qt_}")
            for s in range(S):
                if s == 1:
                    for fn in pend_tail_a:
                        fn()
                    pend_tail_a = []
                if s == 3:
                    for fn in pend_tail_b:
                        fn()
                    pend_tail_b = []
                dist_t = None
                if s > 0:
                    dist_t = distpool.tile([128, P], F32, tag="dist", name=f"d{qt_}_{s}")
                x3 = x_tiles[s][:].rearrange("p (two x) -> p two x", two=2)
                d2ps = []
                for h in range(2):
                    d2p = pp.tile([128, 1024], F32, tag="d2", name=f"d2_{qt_}_{s}_{h}")
                    d2ps.append(d2p)
                    for pc in range(2):
                        o = h * 1024 + pc * 512
                        nc.tensor.matmul(
                            d2p[:, pc * 512 : pc * 512 + 512],
                            lhsT=seedw3,
                            rhs=seedy3[:, :, o : o + 512],
                            start=True,
                            stop=False,
                            perf_mode=DR,
                            skip_group_check=True,
                        )
                        for et in range(ET):
                            nc.tensor.matmul(
                                d2p[:, pc * 512 : pc * 512 + 512],
                                lhsT=x3[:, :, et * QLOC + qt_ * 128 : et * QLOC + qt_ * 128 + 128],
                                rhs=y8d3[:, :, et * P + o : et * P + o + 512],
                                start=False,
                                stop=(et == ET - 1),
                                perf_mode=DR,
                                skip_group_check=True,
                            )
                dst = maccD if s == 0 else dist_t
                qb = qncol[:, qt_ * S + s : qt_ * S + s + 1]
                for h in range(2):
                    hsl = slice(h * 1024, (h + 1) * 1024)
                    if s in SQRT_POOL:
                        # dist = (d2 + qn) ** 0.5 on Pool, freeing ACT
                        nc.gpsimd.tensor_scalar(
                            dst[:, hsl], d2ps[h][:], qb, 0.5, ALU.add, ALU.pow
                        )
                    else:
                        nc.scalar.activation(
                            dst[:, hsl], d2ps[h][:], AF.Sqrt, bias=qb
                        )
                if s > 0:
                    # macc += dist, column-split across DVE and Pool
                    nc.vector.tensor_add(
                        maccD[:, 0:DSPL], maccD[:, 0:DSPL], dist_t[:, 0:DSPL]
                    )
                    nc.gpsimd.tensor_add(
                        maccD[:, DSPL:P], maccD[:, DSPL:P], dist_t[:, DSPL:P]
                    )

            def mk_tail_a(qt_c, maccD_c):
                def emit():
                    m2_t = finpool.tile([128, P], F32, tag="fin", name=f"m2{qt_c}")
                    nc.vector.tensor_mul(m2_t[:], maccD_c[:], maccD_c[:])
                    omean_t = outpool.tile([128, P], F32, tag="out", name=f"om{qt_c}")
                    nc.vector.tensor_scalar_mul(omean_t[:], maccD_c[:], 1.0 / S)
                    nc.sync.dma_start(
                        mean_o[qt_c * 128 : (qt_c + 1) * 128, :], omean_t[:]
                    )
                    return m2_t
                return emit

            def mk_tail_b(qt_c, m2_box):
                def emit():
                    # ss = qnsum + 10*pn + xsum.y via exact fp16 matmuls
                    # (qnsum rides the final Sqrt's bias)
                    ssp = pp.tile([128, P], F32, tag="acc", bufs=1, name=f"ss{qt_c}")
                    for pc in range(4):
                        o = pc * 512
                        nc.tensor.matmul(
                            ssp[:, o : o + 512],
                            lhsT=ones16_t[:],
                            rhs=pn10_t[:, o : o + 512],
                            start=True,
                            stop=False,
                            skip_group_check=True,
                        )
                        for et in range(ET):
                            nc.tensor.matmul(
                                ssp[:, o : o + 512],
                                lhsT=xsum16[:, et * QLOC + qt_c * 128 : et * QLOC + qt_c * 128 + 128],
                                rhs=y16_t[:, et * P + o : et * P + o + 512],
                                start=False,
                                stop=(et == ET - 1),
                                skip_group_check=True,
                            )
                    m2_t = m2_box[0]
                    u_t = finpool.tile([128, P], F32, tag="fin", name=f"u{qt_c}")
                    nc.vector.scalar_tensor_tensor(
                        u_t[:], m2_t[:], -1.0 / S, ssp[:], ALU.mult, ALU.add
                    )
                    ostd_t = outpool.tile([128, P], F32, tag="out", name=f"os{qt_c}")
                    nc.scalar.activation(
                        ostd_t[:], u_t[:], AF.Sqrt,
                        bias=qn9[:, qt_c : qt_c + 1],
                        scale=1.0 / (S - 1),
                    )
                    nc.sync.dma_start(
                        std_o[qt_c * 128 : (qt_c + 1) * 128, :], ostd_t[:]
                    )
                return emit

            m2_box = [None]
            ta = mk_tail_a(qt_, maccD)

            def mk_a(ta_fn, box):
                def emit():
                    box[0] = ta_fn()
                return emit

            pend_tail_a = [mk_a(ta, m2_box)]
            pend_tail_b = [mk_tail_b(qt_, m2_box)]
        for fn in pend_tail_a:
            fn()
        for fn in pend_tail_b:
            fn()


def _prep_inputs(query_features, prototypes, weight_mu, weight_rho, bias_mu, bias_rho, eps_w, eps_b):
    f32, f16 = np.float32, np.float16
    sp_w = np.log1p(np.exp(weight_rho.astype(np.float64))).astype(f32)
    sp_b = np.log1p(np.exp(bias_rho.astype(np.float64))).astype(f32)
    W = (weight_mu[None] + eps_w * sp_w[None]).astype(f32)  # [S,D,D]
    B = (bias_mu[None] + eps_b * sp_b[None]).astype(f32)  # [S,D]
    b2 = (-2.0 * B).astype(f32)  # [S,D]

    y8 = prototypes.astype(f32).astype(E4)  # [P,D] fp8 prototypes
    y8f = y8.astype(f32)
    pn = (y8f.astype(np.float64) ** 2).sum(-1).astype(f32)  # [P]
    pn_hi = (pn * 0.5).astype(E4)
    pn_lo = (pn - 2.0 * pn_hi.astype(f32)).astype(f32).astype(E4)
    pn_seed = (2.0 * pn_hi.astype(f32) + pn_lo.astype(f32)).astype(f32)
    pn10_16 = (float(S) * pn_seed).astype(f16)[None, :]  # [1,P]

    W32 = np.ascontiguousarray(
        W.reshape(S, DT, 128, 256).transpose(0, 2, 1, 3).reshape(S, 128, DT * 256)
    )
    b2T = np.ascontiguousarray(
        b2.T.reshape(ET, 128, S).transpose(1, 0, 2).reshape(128, ET * S)
    )
    yT8 = np.ascontiguousarray(
        y8.T.reshape(ET, 128, P).transpose(1, 0, 2).reshape(128, ET * P)
    )
    y8dup = np.concatenate([yT8, yT8], axis=1)  # [128, 2*ET*P]
    yT16 = yT8.astype(f16)  # exact fp8 -> fp16
    seedw8 = np.concatenate(
        [np.full((1, 128), 2.0, E4), np.full((1, 128), 1.0, E4)], axis=1
    )
    seedy8 = np.concatenate([pn_hi[None, :], pn_lo[None, :]], axis=1)  # [1,2P]
    eye8dup = np.concatenate([np.eye(128, dtype=E4)] * 2, axis=1)
    sel16 = np.zeros((128, S * 16), f16)
    for s in range(S):
        sel16[:, s * 16 + s] = 1.0

    common = {
        "W32": W32,
        "b2T": b2T,
        "y8dup": np.ascontiguousarray(y8dup),
        "yT16": np.ascontiguousarray(yT16),
        "seedw8": seedw8,
        "seedy8": np.ascontiguousarray(seedy8),
        "pn10_16": pn10_16,
        "ones16": np.ones((1, 128), f16),
        "eye8dup": np.ascontiguousarray(eye8dup),
        "eyeT10": np.eye(10, dtype=f32),
        "sel16": sel16,
    }
    qf = query_features.astype(f32)
    in_maps = []
    for c in range(NCORES):
        qs = qf[c * QLOC : (c + 1) * QLOC]  # [QLOC, D]
        qT32 = np.ascontiguousarray(
            qs.T.reshape(DT, 128, QLOC).transpose(1, 0, 2).reshape(128, DT * QLOC)
        )
        in_maps.append({"qT32": qT32, **common})
    return in_maps


def kernel(**inputs):
    global LAST_RESULTS
    n_samples = int(inputs.pop("n_samples", S))
    assert n_samples == S, f"kernel hardcodes S={S}, got {n_samples}"
    np_inputs = {
        k: np.asarray(v, dtype=np.float32)
        for k, v in inputs.items()
    }
    in_maps = _prep_inputs(**np_inputs)

    if "nc" not in _CACHE:
        _CACHE["nc"] = _build_bass()
    nc = _CACHE["nc"]

    trace = bool(int(os.environ.get("KERNEL_TRACE", "0")))
    res = bass_utils.run_bass_kernel_spmd(
        nc, in_maps, core_ids=list(range(NCORES)), trace=trace
    )
    LAST_RESULTS = res
    mean = np.concatenate([r["mean_o"] for r in res.results], axis=0)
    std = np.concatenate([r["std_o"] for r in res.results], axis=0)
    return mean, std


# revision 30
# speedup vs baseline: 1.3242x; 1.3242x over previous
"""Bayesian uncertainty distance kernel for TRN2 (8 NeuronCores, SPMD).

Math (per reference):
    W_s  = weight_mu + eps_w[s] * softplus(weight_rho)          [S,D,D]
    b_s  = bias_mu   + eps_b[s] * softplus(bias_rho)            [S,D]
    qt_s = query @ W_s + b_s                                    [S,Q,D]
    d2_s = ||qt_s||^2 - 2 qt_s.proto^T + ||proto||^2            [S,Q,P]
    mean = mean_s sqrt(d2_s);  std = std_s(sqrt(d2_s), ddof=1)

Sharding: data-parallel over Q (8192 -> 8 x 1024). Everything else replicated.

Design (v2, fp8 DoubleRow):
  - x_s := -2*qt_s - 2*b_s is split into fp8(e4m3) hi+lo pieces
    (xh = e4m3(fp16(x)), xl = e4m3(x - xh)) stored interleaved so the PE
    contracts x~ = xh+xl against fp8 prototypes y~ with DoubleRow matmuls
    (0.5 cyc/col): cross needs 2 DR MMs per 512-col chunk instead of 2
    fp16 passes -> 2x PE.  y~'s rounding is constant across samples so it
    cancels exactly in the sample std; x~'s hi+lo error is ~fp13.
  - ||qt||^2 enters via the ACT Sqrt's per-partition bias (qn columns),
    built by an all-ones select matmul over x2=fp16(x^2) plus a tiny PE
    transpose; pn enters as a rank-1 fp8 DR seed (pn split hi+lo with a
    [2;1]-valued stationary row pair, since e4m3 max is 240 < max pn).
  - Variance via the closed form sum_s d2 = qnsum + 10*pn + xsum.y~ with
    all matmuls in float32r (1 cyc/col >=256 cols, numerically exact
    fp32): xsum accumulated on-PE with fp8 identity-pair DR matmuls so it
    matches sum_s (xh_s+xl_s) bit-exactly -> first-order rounding cancels
    in u = ss - macc^2/S.  qnsum rides the final Sqrt's bias (qnsum/9).
  - macc = sum_s dist: split across engines (DVE adds, Pool adds, and PE
    float32r identity matmuls into PSUM) to balance the four queues; the
    per-sample sqrt stays on ACT (its table op) with dist written once.

The host does only O(S*D^2) prep in numpy (softplus, W_s, transposes, pn).
"""

import os
import numpy as np
import ml_dtypes

import concourse.bass as bass
import concourse.mybir as mybir
import concourse.tile as tile
from concourse import bacc, bass_utils

AF = mybir.ActivationFunctionType
ALU = mybir.AluOpType
DR = mybir.MatmulPerfMode.DoubleRow

F32 = mybir.dt.float32
F32R = mybir.dt.float32r
F16 = mybir.dt.float16
F8 = mybir.dt.float8e4
E4 = ml_dtypes.float8_e4m3

NCORES = 8
D = 256
Q_FULL = 8192
P = 2048
S = 10
QLOC = Q_FULL // NCORES  # 1024
ET = D // 128  # 2 e-tiles
DT = D // 128  # 2 d-tiles
QT = QLOC // 128  # 8 q-tiles per core
EQ = ET * QLOC  # 2048: one hi- or lo-plane of x

# tuning knobs: (s, half) sqrt units routed to DVE via (psum+qn)^0.5 pow;
# macc-add engine per sample; PE adds use exact fp32 identity matmuls.
SQRT_DVE = set()  # tensor_scalar pow fails the ISA check; sqrt stays on ACT
ACC_DVE = (1, 2, 3, 4, 7)
ACC_POOL = (5, 6)
ACC_PE = (8, 9)

_CACHE = {}
LAST_RESULTS = None


def _build_bass(num_devices=NCORES, dbg=False):
    nc = bacc.Bacc(
        "TRN2",
        target_bir_lowering=False,
        debug=False,
        num_devices=num_devices,
    )
    ins = {}
    ins["qT32"] = nc.dram_tensor("qT32", [128, DT * QLOC], F32R, kind="ExternalInput").ap()
    ins["W32"] = nc.dram_tensor("W32", [S, 128, DT * 256], F32R, kind="ExternalInput").ap()
    ins["b2T"] = nc.dram_tensor("b2T", [128, ET * S], F32, kind="ExternalInput").ap()
    ins["y8dup"] = nc.dram_tensor("y8dup", [128, 2 * ET * P], F8, kind="ExternalInput").ap()
    ins["yT16"] = nc.dram_tensor("yT16", [128, ET * P], F16, kind="ExternalInput").ap()
    ins["seedw8"] = nc.dram_tensor("seedw8", [1, 256], F8, kind="ExternalInput").ap()
    ins["seedy8"] = nc.dram_tensor("seedy8", [1, 2 * P], F8, kind="ExternalInput").ap()
    ins["pn10_16"] = nc.dram_tensor("pn10_16", [1, P], F16, kind="ExternalInput").ap()
    ins["ones16"] = nc.dram_tensor("ones16", [1, 128], F16, kind="ExternalInput").ap()
    ins["eye32"] = nc.dram_tensor("eye32", [128, 128], F32, kind="ExternalInput").ap()
    ins["eye8dup"] = nc.dram_tensor("eye8dup", [128, 256], F8, kind="ExternalInput").ap()
    ins["eyeT10"] = nc.dram_tensor("eyeT10", [10, 10], F32, kind="ExternalInput").ap()
    ins["sel16"] = nc.dram_tensor("sel16", [128, S * 16], F16, kind="ExternalInput").ap()
    mean_o = nc.dram_tensor("mean_o", [QLOC, P], F32, kind="ExternalOutput").ap()
    std_o = nc.dram_tensor("std_o", [QLOC, P], F32, kind="ExternalOutput").ap()
    dbg_o = None
    if dbg:
        dbg_o = {
            "dbg_xhl": nc.dram_tensor("dbg_xhl", [S, 128, 2 * EQ], F8, kind="ExternalOutput").ap(),
            "dbg_qncol": nc.dram_tensor("dbg_qncol", [128, QT * S], F32, kind="ExternalOutput").ap(),
            "dbg_xsum": nc.dram_tensor("dbg_xsum", [128, EQ], F16, kind="ExternalOutput").ap(),
        }

    with tile.TileContext(nc) as tc:
        _kernel_body(tc, ins, mean_o, std_o, dbg_o)
    nc.compile()
    return nc


def _kernel_body(tc, ins, mean_o, std_o, dbg_o=None):
    nc = tc.nc
    from contextlib import ExitStack

    ctx = ExitStack()
    with ctx:
        cpool = ctx.enter_context(tc.tile_pool(name="consts", bufs=1))
        wpool = ctx.enter_context(tc.tile_pool(name="wpool", bufs=2))
        xmpool = ctx.enter_context(tc.tile_pool(name="xmpool", bufs=3))
        x2pool = ctx.enter_context(tc.tile_pool(name="x2pool", bufs=2))
        xhlpool = ctx.enter_context(tc.tile_pool(name="xhlpool", bufs=S))
        qnpool = ctx.enter_context(tc.tile_pool(name="qnpool", bufs=1))
        xsumpool = ctx.enter_context(tc.tile_pool(name="xsumpool", bufs=1))
        distpool = ctx.enter_context(tc.tile_pool(name="distpool", bufs=2))
        maccpool = ctx.enter_context(tc.tile_pool(name="maccpool", bufs=2))
        finpool = ctx.enter_context(tc.tile_pool(name="finpool", bufs=2))
        outpool = ctx.enter_context(tc.tile_pool(name="outpool", bufs=2))
        pp = ctx.enter_context(tc.tile_pool(name="pp", bufs=2, space="PSUM"))

        # ---- constants into SBUF ----
        qT_t = cpool.tile([128, DT * QLOC], F32R)
        nc.sync.dma_start(qT_t[:], ins["qT32"])
        b2_t = cpool.tile([128, ET * S], F32)
        nc.sync.dma_start(b2_t[:], ins["b2T"])
        y8_t = cpool.tile([128, 2 * ET * P], F8)
        nc.sync.dma_start(y8_t[:], ins["y8dup"])
        y16_t = cpool.tile([128, ET * P], F16)
        nc.sync.dma_start(y16_t[:], ins["yT16"])
        seedw_t = cpool.tile([1, 256], F8)
        nc.sync.dma_start(seedw_t[:], ins["seedw8"])
        seedy_t = cpool.tile([1, 2 * P], F8)
        nc.sync.dma_start(seedy_t[:], ins["seedy8"])
        pn10_t = cpool.tile([1, P], F16)
        nc.sync.dma_start(pn10_t[:], ins["pn10_16"])
        ones16_t = cpool.tile([1, 128], F16)
        nc.sync.dma_start(ones16_t[:], ins["ones16"])
        eye32_t = cpool.tile([128, 128], F32)
        nc.sync.dma_start(eye32_t[:], ins["eye32"])
        eye8d_t = cpool.tile([128, 256], F8)
        nc.sync.dma_start(eye8d_t[:], ins["eye8dup"])
        eyeT10_t = cpool.tile([10, 10], F32)
        nc.sync.dma_start(eyeT10_t[:], ins["eyeT10"])
        sel_t = cpool.tile([128, S * 16], F16)
        nc.sync.dma_start(sel_t[:], ins["sel16"])

        # qn machinery outputs
        qn_sq = qnpool.tile([10, QLOC], F32)  # qn rows, one partition per s
        qncol = qnpool.tile([128, QT * S], F32)  # qn columns per (qt, s)
        qn9 = qnpool.tile([128, QT], F32)  # qnsum/9 per qt (std bias)
        xsum16 = xsumpool.tile([128, EQ], F16)  # fp16 xsum for the exact ss MMs

        # 3D views for DoubleRow k-tile pairs
        seedw3 = seedw_t[:].rearrange("p (two m) -> p two m", two=2)
        seedy3 = seedy_t[:].rearrange("p (two x) -> p two x", two=2)
        eye8d3 = eye8d_t[:].rearrange("p (two m) -> p two m", two=2)
        y8d3 = y8_t[:].rearrange("p (two x) -> p two x", two=2)

        x_tiles = []
        # ---------- phase 1: per-sample transformed queries ----------
        qnp = None
        for s in range(S):
            w_t = wpool.tile([128, DT * 256], F32R, tag="w", name=f"w{s}")
            nc.sync.dma_start(w_t[:], ins["W32"][s])
            xhl_t = xhlpool.tile([128, 2 * EQ], F8, tag="x", name=f"x{s}")
            x_tiles.append(xhl_t)
            x2_t = x2pool.tile([128, EQ], F16, tag="x2", name=f"x2_{s}")
            for et in range(ET):
                for qc in range(2):
                    qp = pp.tile([128, 512], F32, tag="d2", name=f"qp{s}_{et}_{qc}")
                    for dt_ in range(DT):
                        nc.tensor.matmul(
                            qp[:],
                            lhsT=w_t[:, dt_ * 256 + et * 128 : dt_ * 256 + et * 128 + 128],
                            rhs=qT_t[:, dt_ * QLOC + qc * 512 : dt_ * QLOC + qc * 512 + 512],
                            start=(dt_ == 0),
                            stop=(dt_ == DT - 1),
                        )
                    o = et * QLOC + qc * 512
                    xm = xmpool.tile([128, 512], F16, tag="xm", name=f"xm{s}_{et}_{qc}")
                    # x = -2*qt - 2*b (fp16) on ACT (Identity with bias AP)
                    nc.scalar.activation(
                        xm[:], qp[:], AF.Identity,
                        bias=b2_t[:, et * S + s : et * S + s + 1],
                        scale=-2.0,
                    )
                    # hi piece: fp8 round of x (DVE tensor_scalar convert:
                    # InstCast gets no fast path, TensorScalarPtr does)
                    nc.vector.tensor_scalar_add(xhl_t[:, o : o + 512], xm[:], 0.0)
                    # lo piece: fp8(x - xh), split DVE/Pool to balance
                    if (et + qc) % 2 == 0:
                        nc.gpsimd.tensor_sub(
                            xhl_t[:, EQ + o : EQ + o + 512], xm[:], xhl_t[:, o : o + 512]
                        )
                    else:
                        nc.vector.tensor_sub(
                            xhl_t[:, EQ + o : EQ + o + 512], xm[:], xhl_t[:, o : o + 512]
                        )
                    # x2 = x^2 fp16 on DVE (2-byte fast path)
                    nc.vector.tensor_mul(x2_t[:, o : o + 512], xm[:], xm[:])
            # qn rows: select-matmul accumulates 0.25*sum_e x2 into psum row s
            if s == 0:
                qnp = pp.tile([10, QLOC], F32, tag="acc", bufs=1, name="qnp")
            for qc in range(2):
                for et in range(ET):
                    nc.tensor.matmul(
                        qnp[:, qc * 512 : qc * 512 + 512],
                        lhsT=sel_t[:, s * 16 : s * 16 + 10],
                        rhs=x2_t[:, et * QLOC + qc * 512 : et * QLOC + qc * 512 + 512],
                        start=(s == 0 and et == 0),
                        stop=(s == S - 1 and et == ET - 1),
                        skip_group_check=True,
                    )

        # qn rows -> sbuf (scale 0.25 compensates x=(2qt+2b): qn=||x/2||^2)
        nc.scalar.activation(qn_sq[:, :], qnp[:, :], AF.Copy, scale=0.25)
        # transpose 128-blocks to get per-(qt,s) bias columns
        for qt_ in range(QT):
            qtp = pp.tile([128, 10], F32, tag="d2", name=f"qtp{qt_}")
            nc.tensor.matmul(
                qtp[:],
                lhsT=qn_sq[0:10, qt_ * 128 : qt_ * 128 + 128],
                rhs=eyeT10_t[:],
                is_transpose=True,
            )
            nc.scalar.copy(qncol[:, qt_ * S : qt_ * S + S], qtp[:])
            nc.vector.tensor_reduce(
                qn9[:, qt_ : qt_ + 1],
                qncol[:, qt_ * S : qt_ * S + S],
                axis=mybir.AxisListType.X,
                op=ALU.add,
            )
        nc.vector.tensor_scalar_mul(qn9[:], qn9[:], 1.0 / (S - 1))

        # xsum = sum_s (xh_s + xl_s) via fp8 identity-pair DR matmuls
        xsp = pp.tile([128, EQ], F32, tag="acc", bufs=1, name="xsp")
        for s in range(S):
            x3 = x_tiles[s][:].rearrange("p (two x) -> p two x", two=2)
            for et in range(ET):
                for qc in range(2):
                    o = et * QLOC + qc * 512
                    nc.tensor.matmul(
                        xsp[:, o : o + 512],
                        lhsT=eye8d3,
                        rhs=x3[:, :, o : o + 512],
                        start=(s == 0),
                        stop=(s == S - 1),
                        perf_mode=DR,
                        skip_group_check=True,
                    )
        nc.vector.tensor_copy(xsum16[:, 0:QLOC], xsp[:, 0:QLOC])
        nc.scalar.copy(xsum16[:, QLOC:EQ], xsp[:, QLOC:EQ])

        if dbg_o is not None:
            for s in range(S):
                nc.sync.dma_start(dbg_o["dbg_xhl"][s], x_tiles[s][:])
            nc.sync.dma_start(dbg_o["dbg_qncol"], qncol[:])
            nc.sync.dma_start(dbg_o["dbg_xsum"], xsum16[:])

        # ---------- phase 2: distances, moments, outputs ----------
        # Tail work of qt_ is deferred into qt_+1's s-loop (stage A at s==1,
        # stage B at s==3) so the PE queue never stalls on not-yet-emitted
        # ACT/DVE work.
        pend_tail_a = []
        pend_tail_b = []
        for qt_ in range(QT):
            maccD = maccpool.tile([128, P], F32, tag="maccD", name=f"maccD{qt_}")
            maccP = None
            pend_pe = []
            for s in range(S):
                if s == 1:
                    for fn in pend_tail_a:
                        fn()
                    pend_tail_a = []
                if s == 3:
                    for fn in pend_tail_b:
                        fn()
                    pend_tail_b = []
                dist_t = None
                if s > 0:
                    dist_t = distpool.tile([128, P], F32, tag="dist", name=f"d{qt_}_{s}")
                x3 = x_tiles[s][:].rearrange("p (two x) -> p two x", two=2)
                d2ps = []
                for h in range(2):
                    d2p = pp.tile([128, 1024], F32, tag="d2", name=f"d2_{qt_}_{s}_{h}")
                    d2ps.append(d2p)
                    for pc in range(2):
                        o = h * 1024 + pc * 512
                        nc.tensor.matmul(
                            d2p[:, pc * 512 : pc * 512 + 512],
                            lhsT=seedw3,
                            rhs=seedy3[:, :, o : o + 512],
                            start=True,
                            stop=False,
                            perf_mode=DR,
                            skip_group_check=True,
                        )
                        for et in range(ET):
                            nc.tensor.matmul(
                                d2p[:, pc * 512 : pc * 512 + 512],
                                lhsT=x3[:, :, et * QLOC + qt_ * 128 : et * QLOC + qt_ * 128 + 128],
                                rhs=y8d3[:, :, et * P + o : et * P + o + 512],
                                start=False,
                                stop=(et == ET - 1),
                                perf_mode=DR,
                                skip_group_check=True,
                            )
                # PE-side accumulation of the previous dist (lag keeps PE
                # dense: these matmuls sit behind the next sample's d2 MMs)
                for fn in pend_pe:
                    fn()
                pend_pe = []
                dst = maccD if s == 0 else dist_t
                qb = qncol[:, qt_ * S + s : qt_ * S + s + 1]
                for h in range(2):
                    hsl = slice(h * 1024, (h + 1) * 1024)
                    if (s, h) in SQRT_DVE:
                        # dist = (d2 + qn) ** 0.5 on DVE, freeing ACT
                        nc.vector.tensor_scalar(
                            dst[:, hsl], d2ps[h][:], qb, 0.5, ALU.add, ALU.pow
                        )
                    else:
                        nc.scalar.activation(
                            dst[:, hsl], d2ps[h][:], AF.Sqrt, bias=qb
                        )
                if s in ACC_DVE:
                    nc.vector.tensor_add(maccD[:], maccD[:], dist_t[:])
                elif s in ACC_POOL:
                    nc.gpsimd.tensor_add(maccD[:], maccD[:], dist_t[:])
                elif s in ACC_PE:
                    if s == ACC_PE[0]:
                        maccP = pp.tile(
                            [128, P], F32, tag="acc", bufs=1, name=f"maccP{qt_}"
                        )

                    def mk_acc(mp, dt_loc, first, last):
                        def emit():
                            for pc in range(4):
                                nc.tensor.matmul(
                                    mp[:, pc * 512 : pc * 512 + 512],
                                    lhsT=eye32_t[:],
                                    rhs=dt_loc[:, pc * 512 : pc * 512 + 512],
                                    start=first,
                                    stop=last,
                                    skip_group_check=True,
                                )
                        return emit
                    pend_pe.append(
                        mk_acc(maccP, dist_t, s == ACC_PE[0], s == ACC_PE[-1])
                    )

            def mk_tail_a(qt_c, maccD_c, maccP_c, pe_fns):
                def emit():
                    for fn in pe_fns:
                        fn()
                    # macc = maccD + maccP (single PSUM operand is allowed)
                    nc.vector.tensor_add(maccD_c[:], maccD_c[:], maccP_c[:])
                    m2_t = finpool.tile([128, P], F32, tag="fin", name=f"m2{qt_c}")
                    nc.vector.tensor_mul(m2_t[:], maccD_c[:], maccD_c[:])
                    omean_t = outpool.tile([128, P], F32, tag="out", name=f"om{qt_c}")
                    nc.vector.tensor_scalar_mul(omean_t[:], maccD_c[:], 1.0 / S)
                    nc.sync.dma_start(
                        mean_o[qt_c * 128 : (qt_c + 1) * 128, :], omean_t[:]
                    )
                    return m2_t
                return emit

            def mk_tail_b(qt_c, m2_box):
                def emit():
                    # ss = qnsum + 10*pn + xsum.y via exact fp16 matmuls
                    # (qnsum rides the final Sqrt's bias)
                    ssp = pp.tile([128, P], F32, tag="acc", bufs=1, name=f"ss{qt_c}")
                    for pc in range(4):
                        o = pc * 512
                        nc.tensor.matmul(
                            ssp[:, o : o + 512],
                            lhsT=ones16_t[:],
                            rhs=pn10_t[:, o : o + 512],
                            start=True,
                            stop=False,
                            skip_group_check=True,
                        )
                        for et in range(ET):
                            nc.tensor.matmul(
                                ssp[:, o : o + 512],
                                lhsT=xsum16[:, et * QLOC + qt_c * 128 : et * QLOC + qt_c * 128 + 128],
                                rhs=y16_t[:, et * P + o : et * P + o + 512],
                                start=False,
                                stop=(et == ET - 1),
                                skip_group_check=True,
                            )
                    m2_t = m2_box[0]
                    u_t = finpool.tile([128, P], F32, tag="fin", name=f"u{qt_c}")
                    nc.vector.scalar_tensor_tensor(
                        u_t[:], m2_t[:], -1.0 / S, ssp[:], ALU.mult, ALU.add
                    )
                    ostd_t = outpool.tile([128, P], F32, tag="out", name=f"os{qt_c}")
                    nc.scalar.activation(
                        ostd_t[:], u_t[:], AF.Sqrt,
                        bias=qn9[:, qt_c : qt_c + 1],
                        scale=1.0 / (S - 1),
                    )
                    nc.sync.dma_start(
                        std_o[qt_c * 128 : (qt_c + 1) * 128, :], ostd_t[:]
                    )
                return emit

            m2_box = [None]
            ta = mk_tail_a(qt_, maccD, maccP, list(pend_pe))
            pend_pe = []

            def mk_a(ta_fn, box):
                def emit():
                    box[0] = ta_fn()
                return emit

            pend_tail_a = [mk_a(ta, m2_box)]
            pend_tail_b = [mk_tail_b(qt_, m2_box)]
        for fn in pend_tail_a:
            fn()
        for fn in pend_tail_b:
            fn()


def _prep_inputs(query_features, prototypes, weight_mu, weight_rho, bias_mu, bias_rho, eps_w, eps_b):
    f32, f16 = np.float32, np.float16
    sp_w = np.log1p(np.exp(weight_rho.astype(np.float64))).astype(f32)
    sp_b = np.log1p(np.exp(bias_rho.astype(np.float64))).astype(f32)
    W = (weight_mu[None] + eps_w * sp_w[None]).astype(f32)  # [S,D,D]
    B = (bias_mu[None] + eps_b * sp_b[None]).astype(f32)  # [S,D]
    b2 = (-2.0 * B).astype(f32)  # [S,D]

    y8 = prototypes.astype(f32).astype(E4)  # [P,D] fp8 prototypes
    y8f = y8.astype(f32)
    pn = (y8f.astype(np.float64) ** 2).sum(-1).astype(f32)  # [P]
    pn_hi = (pn * 0.5).astype(E4)
    pn_lo = (pn - 2.0 * pn_hi.astype(f32)).astype(f32).astype(E4)
    pn_seed = (2.0 * pn_hi.astype(f32) + pn_lo.astype(f32)).astype(f32)
    pn10_16 = (float(S) * pn_seed).astype(f16)[None, :]  # [1,P]

    W32 = np.ascontiguousarray(
        W.reshape(S, DT, 128, 256).transpose(0, 2, 1, 3).reshape(S, 128, DT * 256)
    )
    b2T = np.ascontiguousarray(
        b2.T.reshape(ET, 128, S).transpose(1, 0, 2).reshape(128, ET * S)
    )
    yT8 = np.ascontiguousarray(
        y8.T.reshape(ET, 128, P).transpose(1, 0, 2).reshape(128, ET * P)
    )
    y8dup = np.concatenate([yT8, yT8], axis=1)  # [128, 2*ET*P]
    yT16 = yT8.astype(f16)  # exact fp8 -> fp16
    seedw8 = np.concatenate(
        [np.full((1, 128), 2.0, E4), np.full((1, 128), 1.0, E4)], axis=1
    )
    seedy8 = np.concatenate([pn_hi[None, :], pn_lo[None, :]], axis=1)  # [1,2P]
    eye8dup = np.concatenate([np.eye(128, dtype=E4)] * 2, axis=1)
    sel16 = np.zeros((128, S * 16), f16)
    for s in range(S):
        sel16[:, s * 16 + s] = 1.0

    common = {
        "W32": W32,
        "b2T": b2T,
        "y8dup": np.ascontiguousarray(y8dup),
        "yT16": np.ascontiguousarray(yT16),
        "seedw8": seedw8,
        "seedy8": np.ascontiguousarray(seedy8),
        "pn10_16": pn10_16,
        "ones16": np.ones((1, 128), f16),
        "eye32": np.eye(128, dtype=f32),
        "eye8dup": np.ascontiguousarray(eye8dup),
        "eyeT10": np.eye(10, dtype=f32),
        "sel16": sel16,
    }
    qf = query_features.astype(f32)
    in_maps = []
    for c in range(NCORES):
        qs = qf[c * QLOC : (c + 1) * QLOC]  # [QLOC, D]
        qT32 = np.ascontiguousarray(
            qs.T.reshape(DT, 128, QLOC).transpose(1, 0, 2).reshape(128, DT * QLOC)
        )
        in_maps.append({"qT32": qT32, **common})
    return in_maps


def kernel(**inputs):
    global LAST_RESULTS
    n_samples = int(inputs.pop("n_samples", S))
    assert n_samples == S, f"kernel hardcodes S={S}, got {n_samples}"
    np_inputs = {
        k: np.asarray(v, dtype=np.float32)
        for k, v in inputs.items()
    }
    in_maps = _prep_inputs(**np_inputs)

    if "nc" not in _CACHE:
        _CACHE["nc"] = _build_bass()
    nc = _CACHE["nc"]

    trace = bool(int(os.environ.get("KERNEL_TRACE", "0")))
    res = bass_utils.run_bass_kernel_spmd(
        nc, in_maps, core_ids=list(range(NCORES)), trace=trace
    )
    LAST_RESULTS = res
    mean = np.concatenate([r["mean_o"] for r in res.results], axis=0)
    std = np.concatenate([r["std_o"] for r in res.results], axis=0)
    return mean, std


# revision 33
# speedup vs baseline: 1.5595x; 1.1777x over previous
"""Bayesian uncertainty distance kernel for TRN2 (8 NeuronCores, SPMD).

Math (per reference):
    W_s  = weight_mu + eps_w[s] * softplus(weight_rho)          [S,D,D]
    b_s  = bias_mu   + eps_b[s] * softplus(bias_rho)            [S,D]
    qt_s = query @ W_s + b_s                                    [S,Q,D]
    d2_s = ||qt_s||^2 - 2 qt_s.proto^T + ||proto||^2            [S,Q,P]
    mean = mean_s sqrt(d2_s);  std = std_s(sqrt(d2_s), ddof=1)

Sharding: data-parallel over Q (8192 -> 8 x 1024). Everything else replicated.

Design (v2, fp8 DoubleRow):
  - x_s := -2*qt_s - 2*b_s is split into fp8(e4m3) hi+lo pieces
    (xh = e4m3(fp16(x)), xl = e4m3(x - xh)) stored interleaved so the PE
    contracts x~ = xh+xl against fp8 prototypes y~ with DoubleRow matmuls
    (0.5 cyc/col): cross needs 2 DR MMs per 512-col chunk instead of 2
    fp16 passes -> 2x PE.  y~'s rounding is constant across samples so it
    cancels exactly in the sample std; x~'s hi+lo error is ~fp13.
  - ||qt||^2 enters via the ACT Sqrt's per-partition bias (qn columns),
    built by an all-ones select matmul over x2=fp16(x^2) plus a tiny PE
    transpose; pn enters as a rank-1 fp8 DR seed (pn split hi+lo with a
    [2;1]-valued stationary row pair, since e4m3 max is 240 < max pn).
  - Variance via the closed form sum_s d2 = qnsum + 10*pn + xsum.y~ with
    all matmuls in float32r (1 cyc/col >=256 cols, numerically exact
    fp32): xsum accumulated on-PE with fp8 identity-pair DR matmuls so it
    matches sum_s (xh_s+xl_s) bit-exactly -> first-order rounding cancels
    in u = ss - macc^2/S.  qnsum rides the final Sqrt's bias (qnsum/9).
  - macc = sum_s dist: split across engines (DVE adds, Pool adds, and PE
    float32r identity matmuls into PSUM) to balance the four queues; the
    per-sample sqrt stays on ACT (its table op) with dist written once.

The host does only O(S*D^2) prep in numpy (softplus, W_s, transposes, pn).
"""

import os
import numpy as np
import ml_dtypes

import concourse.bass as bass
import concourse.mybir as mybir
import concourse.tile as tile
from concourse import bacc, bass_utils

AF = mybir.ActivationFunctionType
ALU = mybir.AluOpType
DR = mybir.MatmulPerfMode.DoubleRow

F32 = mybir.dt.float32
F32R = mybir.dt.float32r
F16 = mybir.dt.float16
F8 = mybir.dt.float8e4
E4 = ml_dtypes.float8_e4m3

NCORES = 8
D = 256
Q_FULL = 8192
P = 2048
S = 10
QLOC = Q_FULL // NCORES  # 1024
ET = D // 128  # 2 e-tiles
DT = D // 128  # 2 d-tiles
QT = QLOC // 128  # 8 q-tiles per core
EQ = ET * QLOC  # 2048: one hi- or lo-plane of x

# tuning knobs: (s, half) sqrt units routed to DVE via (psum+qn)^0.5 pow;
# macc-add engine per sample; PE adds use exact fp32 identity matmuls.
SQRT_DVE = set()  # tensor_scalar pow fails the ISA check; sqrt stays on ACT
ACC_DVE = (1, 2, 3, 4, 7, 8)
ACC_POOL = (5, 6, 9)
ACC_PE = ()

_CACHE = {}
LAST_RESULTS = None


def _build_bass(num_devices=NCORES, dbg=False):
    nc = bacc.Bacc(
        "TRN2",
        target_bir_lowering=False,
        debug=False,
        num_devices=num_devices,
    )
    ins = {}
    ins["qT32"] = nc.dram_tensor("qT32", [128, DT * QLOC], F32R, kind="ExternalInput").ap()
    ins["W32"] = nc.dram_tensor("W32", [S, 128, DT * 256], F32R, kind="ExternalInput").ap()
    ins["b2T"] = nc.dram_tensor("b2T", [128, ET * S], F32, kind="ExternalInput").ap()
    ins["y8dup"] = nc.dram_tensor("y8dup", [128, 2 * ET * P], F8, kind="ExternalInput").ap()
    ins["yT16"] = nc.dram_tensor("yT16", [128, ET * P], F16, kind="ExternalInput").ap()
    ins["seedw8"] = nc.dram_tensor("seedw8", [1, 256], F8, kind="ExternalInput").ap()
    ins["seedy8"] = nc.dram_tensor("seedy8", [1, 2 * P], F8, kind="ExternalInput").ap()
    ins["pn10_16"] = nc.dram_tensor("pn10_16", [1, P], F16, kind="ExternalInput").ap()
    ins["ones16"] = nc.dram_tensor("ones16", [1, 128], F16, kind="ExternalInput").ap()
    ins["eye32"] = nc.dram_tensor("eye32", [128, 128], F32, kind="ExternalInput").ap()
    ins["eye8dup"] = nc.dram_tensor("eye8dup", [128, 256], F8, kind="ExternalInput").ap()
    ins["eyeT10"] = nc.dram_tensor("eyeT10", [10, 10], F32, kind="ExternalInput").ap()
    ins["sel16"] = nc.dram_tensor("sel16", [128, S * 16], F16, kind="ExternalInput").ap()
    mean_o = nc.dram_tensor("mean_o", [QLOC, P], F32, kind="ExternalOutput").ap()
    std_o = nc.dram_tensor("std_o", [QLOC, P], F32, kind="ExternalOutput").ap()
    dbg_o = None
    if dbg:
        dbg_o = {
            "dbg_xhl": nc.dram_tensor("dbg_xhl", [S, 128, 2 * EQ], F8, kind="ExternalOutput").ap(),
            "dbg_qncol": nc.dram_tensor("dbg_qncol", [128, QT * S], F32, kind="ExternalOutput").ap(),
            "dbg_xsum": nc.dram_tensor("dbg_xsum", [128, EQ], F16, kind="ExternalOutput").ap(),
        }

    with tile.TileContext(nc) as tc:
        _kernel_body(tc, ins, mean_o, std_o, dbg_o)
    nc.compile()
    return nc


def _kernel_body(tc, ins, mean_o, std_o, dbg_o=None):
    nc = tc.nc
    from contextlib import ExitStack

    ctx = ExitStack()
    with ctx:
        cpool = ctx.enter_context(tc.tile_pool(name="consts", bufs=1))
        wpool = ctx.enter_context(tc.tile_pool(name="wpool", bufs=2))
        xmpool = ctx.enter_context(tc.tile_pool(name="xmpool", bufs=3))
        x2pool = ctx.enter_context(tc.tile_pool(name="x2pool", bufs=2))
        xhlpool = ctx.enter_context(tc.tile_pool(name="xhlpool", bufs=S))
        qnpool = ctx.enter_context(tc.tile_pool(name="qnpool", bufs=1))
        xsumpool = ctx.enter_context(tc.tile_pool(name="xsumpool", bufs=1))
        distpool = ctx.enter_context(tc.tile_pool(name="distpool", bufs=3))
        maccpool = ctx.enter_context(tc.tile_pool(name="maccpool", bufs=2))
        finpool = ctx.enter_context(tc.tile_pool(name="finpool", bufs=2))
        outpool = ctx.enter_context(tc.tile_pool(name="outpool", bufs=2))
        pp = ctx.enter_context(tc.tile_pool(name="pp", bufs=2, space="PSUM"))

        # ---- constants into SBUF ----
        qT_t = cpool.tile([128, DT * QLOC], F32R)
        nc.sync.dma_start(qT_t[:], ins["qT32"])
        b2_t = cpool.tile([128, ET * S], F32)
        nc.sync.dma_start(b2_t[:], ins["b2T"])
        y8_t = cpool.tile([128, 2 * ET * P], F8)
        nc.sync.dma_start(y8_t[:], ins["y8dup"])
        y16_t = cpool.tile([128, ET * P], F16)
        nc.sync.dma_start(y16_t[:], ins["yT16"])
        seedw_t = cpool.tile([1, 256], F8)
        nc.sync.dma_start(seedw_t[:], ins["seedw8"])
        seedy_t = cpool.tile([1, 2 * P], F8)
        nc.sync.dma_start(seedy_t[:], ins["seedy8"])
        pn10_t = cpool.tile([1, P], F16)
        nc.sync.dma_start(pn10_t[:], ins["pn10_16"])
        ones16_t = cpool.tile([1, 128], F16)
        nc.sync.dma_start(ones16_t[:], ins["ones16"])
        eye32_t = cpool.tile([128, 128], F32)
        nc.sync.dma_start(eye32_t[:], ins["eye32"])
        eye8d_t = cpool.tile([128, 256], F8)
        nc.sync.dma_start(eye8d_t[:], ins["eye8dup"])
        eyeT10_t = cpool.tile([10, 10], F32)
        nc.sync.dma_start(eyeT10_t[:], ins["eyeT10"])
        sel_t = cpool.tile([128, S * 16], F16)
        nc.sync.dma_start(sel_t[:], ins["sel16"])

        # qn machinery outputs
        qn_sq = qnpool.tile([10, QLOC], F32)  # qn rows, one partition per s
        qncol = qnpool.tile([128, QT * S], F32)  # qn columns per (qt, s)
        qn9 = qnpool.tile([128, QT], F32)  # qnsum/9 per qt (std bias)
        xsum16 = xsumpool.tile([128, EQ], F16)  # fp16 xsum for the exact ss MMs

        # 3D views for DoubleRow k-tile pairs
        seedw3 = seedw_t[:].rearrange("p (two m) -> p two m", two=2)
        seedy3 = seedy_t[:].rearrange("p (two x) -> p two x", two=2)
        eye8d3 = eye8d_t[:].rearrange("p (two m) -> p two m", two=2)
        y8d3 = y8_t[:].rearrange("p (two x) -> p two x", two=2)

        x_tiles = []
        # ---------- phase 1: per-sample transformed queries ----------
        qnp = None
        for s in range(S):
            w_t = wpool.tile([128, DT * 256], F32R, tag="w", name=f"w{s}")
            nc.sync.dma_start(w_t[:], ins["W32"][s])
            xhl_t = xhlpool.tile([128, 2 * EQ], F8, tag="x", name=f"x{s}")
            x_tiles.append(xhl_t)
            x2_t = x2pool.tile([128, EQ], F16, tag="x2", name=f"x2_{s}")
            for et in range(ET):
                for qc in range(2):
                    qp = pp.tile([128, 512], F32, tag="d2", name=f"qp{s}_{et}_{qc}")
                    for dt_ in range(DT):
                        nc.tensor.matmul(
                            qp[:],
                            lhsT=w_t[:, dt_ * 256 + et * 128 : dt_ * 256 + et * 128 + 128],
                            rhs=qT_t[:, dt_ * QLOC + qc * 512 : dt_ * QLOC + qc * 512 + 512],
                            start=(dt_ == 0),
                            stop=(dt_ == DT - 1),
                        )
                    o = et * QLOC + qc * 512
                    xm = xmpool.tile([128, 512], F16, tag="xm", name=f"xm{s}_{et}_{qc}")
                    # x = -2*qt - 2*b (fp16) on ACT (Identity with bias AP)
                    nc.scalar.activation(
                        xm[:], qp[:], AF.Identity,
                        bias=b2_t[:, et * S + s : et * S + s + 1],
                        scale=-2.0,
                    )
                    # hi piece: fp8 round of x (DVE tensor_scalar convert:
                    # InstCast gets no fast path, TensorScalarPtr does)
                    nc.vector.tensor_scalar_add(xhl_t[:, o : o + 512], xm[:], 0.0)
                    # lo piece: fp8(x - xh), split DVE/Pool to balance
                    if (et + qc) % 2 == 0:
                        nc.gpsimd.tensor_sub(
                            xhl_t[:, EQ + o : EQ + o + 512], xm[:], xhl_t[:, o : o + 512]
                        )
                    else:
                        nc.vector.tensor_sub(
                            xhl_t[:, EQ + o : EQ + o + 512], xm[:], xhl_t[:, o : o + 512]
                        )
                    # x2 = x^2 fp16 on DVE (2-byte fast path)
                    nc.vector.tensor_mul(x2_t[:, o : o + 512], xm[:], xm[:])
            # qn rows: select-matmul accumulates 0.25*sum_e x2 into psum row s
            if s == 0:
                qnp = pp.tile([10, QLOC], F32, tag="acc", bufs=1, name="qnp")
            for qc in range(2):
                for et in range(ET):
                    nc.tensor.matmul(
                        qnp[:, qc * 512 : qc * 512 + 512],
                        lhsT=sel_t[:, s * 16 : s * 16 + 10],
                        rhs=x2_t[:, et * QLOC + qc * 512 : et * QLOC + qc * 512 + 512],
                        start=(s == 0 and et == 0),
                        stop=(s == S - 1 and et == ET - 1),
                        skip_group_check=True,
                    )

        # qn rows -> sbuf (scale 0.25 compensates x=(2qt+2b): qn=||x/2||^2)
        nc.scalar.activation(qn_sq[:, :], qnp[:, :], AF.Copy, scale=0.25)
        # transpose 128-blocks to get per-(qt,s) bias columns
        for qt_ in range(QT):
            qtp = pp.tile([128, 10], F32, tag="d2", name=f"qtp{qt_}")
            nc.tensor.matmul(
                qtp[:],
                lhsT=qn_sq[0:10, qt_ * 128 : qt_ * 128 + 128],
                rhs=eyeT10_t[:],
                is_transpose=True,
            )
            nc.scalar.copy(qncol[:, qt_ * S : qt_ * S + S], qtp[:])
            nc.vector.tensor_reduce(
                qn9[:, qt_ : qt_ + 1],
                qncol[:, qt_ * S : qt_ * S + S],
                axis=mybir.AxisListType.X,
                op=ALU.add,
            )
        nc.vector.tensor_scalar_mul(qn9[:], qn9[:], 1.0 / (S - 1))

        # xsum = sum_s (xh_s + xl_s) via fp8 identity-pair DR matmuls
        xsp = pp.tile([128, EQ], F32, tag="acc", bufs=1, name="xsp")
        for s in range(S):
            x3 = x_tiles[s][:].rearrange("p (two x) -> p two x", two=2)
            for et in range(ET):
                for qc in range(2):
                    o = et * QLOC + qc * 512
                    nc.tensor.matmul(
                        xsp[:, o : o + 512],
                        lhsT=eye8d3,
                        rhs=x3[:, :, o : o + 512],
                        start=(s == 0),
                        stop=(s == S - 1),
                        perf_mode=DR,
                        skip_group_check=True,
                    )
        nc.vector.tensor_copy(xsum16[:, 0:QLOC], xsp[:, 0:QLOC])
        nc.scalar.copy(xsum16[:, QLOC:EQ], xsp[:, QLOC:EQ])

        if dbg_o is not None:
            for s in range(S):
                nc.sync.dma_start(dbg_o["dbg_xhl"][s], x_tiles[s][:])
            nc.sync.dma_start(dbg_o["dbg_qncol"], qncol[:])
            nc.sync.dma_start(dbg_o["dbg_xsum"], xsum16[:])

        # ---------- phase 2: distances, moments, outputs ----------
        # Tail work of qt_ is deferred into qt_+1's s-loop (stage A at s==1,
        # stage B at s==3) so the PE queue never stalls on not-yet-emitted
        # ACT/DVE work.
        pend_tail_a = []
        pend_tail_b = []
        for qt_ in range(QT):
            maccD = maccpool.tile([128, P], F32, tag="maccD", name=f"maccD{qt_}")
            maccP = None
            pend_pe = []
            for s in range(S):
                if s == 1:
                    for fn in pend_tail_a:
                        fn()
                    pend_tail_a = []
                if s == 3:
                    for fn in pend_tail_b:
                        fn()
                    pend_tail_b = []
                dist_t = None
                if s > 0:
                    dist_t = distpool.tile([128, P], F32, tag="dist", name=f"d{qt_}_{s}")
                x3 = x_tiles[s][:].rearrange("p (two x) -> p two x", two=2)
                d2ps = []
                for h in range(2):
                    d2p = pp.tile([128, 1024], F32, tag="d2", name=f"d2_{qt_}_{s}_{h}")
                    d2ps.append(d2p)
                    for pc in range(2):
                        o = h * 1024 + pc * 512
                        nc.tensor.matmul(
                            d2p[:, pc * 512 : pc * 512 + 512],
                            lhsT=seedw3,
                            rhs=seedy3[:, :, o : o + 512],
                            start=True,
                            stop=False,
                            perf_mode=DR,
                            skip_group_check=True,
                        )
                        for et in range(ET):
                            nc.tensor.matmul(
                                d2p[:, pc * 512 : pc * 512 + 512],
                                lhsT=x3[:, :, et * QLOC + qt_ * 128 : et * QLOC + qt_ * 128 + 128],
                                rhs=y8d3[:, :, et * P + o : et * P + o + 512],
                                start=False,
                                stop=(et == ET - 1),
                                perf_mode=DR,
                                skip_group_check=True,
                            )
                # PE-side accumulation of the previous dist (lag keeps PE
                # dense: these matmuls sit behind the next sample's d2 MMs)
                for fn in pend_pe:
                    fn()
                pend_pe = []
                dst = maccD if s == 0 else dist_t
                qb = qncol[:, qt_ * S + s : qt_ * S + s + 1]
                for h in range(2):
                    hsl = slice(h * 1024, (h + 1) * 1024)
                    if (s, h) in SQRT_DVE:
                        # dist = (d2 + qn) ** 0.5 on DVE, freeing ACT
                        nc.vector.tensor_scalar(
                            dst[:, hsl], d2ps[h][:], qb, 0.5, ALU.add, ALU.pow
                        )
                    else:
                        nc.scalar.activation(
                            dst[:, hsl], d2ps[h][:], AF.Sqrt, bias=qb
                        )
                if s in ACC_DVE:
                    nc.vector.tensor_add(maccD[:], maccD[:], dist_t[:])
                elif s in ACC_POOL:
                    nc.gpsimd.tensor_add(maccD[:], maccD[:], dist_t[:])
                elif s in ACC_PE:
                    if s == ACC_PE[0]:
                        maccP = pp.tile(
                            [128, P], F32, tag="acc", bufs=1, name=f"maccP{qt_}"
                        )

                    def mk_acc(mp, dt_loc, first, last):
                        def emit():
                            for pc in range(4):
                                nc.tensor.matmul(
                                    mp[:, pc * 512 : pc * 512 + 512],
                                    lhsT=eye32_t[:],
                                    rhs=dt_loc[:, pc * 512 : pc * 512 + 512],
                                    start=first,
                                    stop=last,
                                    skip_group_check=True,
                                )
                        return emit
                    pend_pe.append(
                        mk_acc(maccP, dist_t, s == ACC_PE[0], s == ACC_PE[-1])
                    )

            def mk_tail_a(qt_c, maccD_c, maccP_c, pe_fns):
                def emit():
                    for fn in pe_fns:
                        fn()
                    if maccP_c is not None:
                        # macc = maccD + maccP (single PSUM operand is allowed)
                        nc.vector.tensor_add(maccD_c[:], maccD_c[:], maccP_c[:])
                    m2_t = finpool.tile([128, P], F32, tag="fin", name=f"m2{qt_c}")
                    nc.vector.tensor_mul(m2_t[:], maccD_c[:], maccD_c[:])
                    omean_t = outpool.tile([128, P], F32, tag="out", name=f"om{qt_c}")
                    nc.vector.tensor_scalar_mul(omean_t[:], maccD_c[:], 1.0 / S)
                    nc.sync.dma_start(
                        mean_o[qt_c * 128 : (qt_c + 1) * 128, :], omean_t[:]
                    )
                    return m2_t
                return emit

            def mk_tail_b(qt_c, m2_box):
                def emit():
                    # ss = qnsum + 10*pn + xsum.y via exact fp16 matmuls
                    # (qnsum rides the final Sqrt's bias)
                    ssp = pp.tile([128, P], F32, tag="acc", bufs=1, name=f"ss{qt_c}")
                    for pc in range(4):
                        o = pc * 512
                        nc.tensor.matmul(
                            ssp[:, o : o + 512],
                            lhsT=ones16_t[:],
                            rhs=pn10_t[:, o : o + 512],
                            start=True,
                            stop=False,
                            skip_group_check=True,
                        )
                        for et in range(ET):
                            nc.tensor.matmul(
                                ssp[:, o : o + 512],
                                lhsT=xsum16[:, et * QLOC + qt_c * 128 : et * QLOC + qt_c * 128 + 128],
                                rhs=y16_t[:, et * P + o : et * P + o + 512],
                                start=False,
                                stop=(et == ET - 1),
                                skip_group_check=True,
                            )
                    m2_t = m2_box[0]
                    u_t = finpool.tile([128, P], F32, tag="fin", name=f"u{qt_c}")
                    nc.vector.scalar_tensor_tensor(
                        u_t[:], m2_t[:], -1.0 / S, ssp[:], ALU.mult, ALU.add
                    )
                    ostd_t = outpool.tile([128, P], F32, tag="out", name=f"os{qt_c}")
                    nc.scalar.activation(
                        ostd_t[:], u_t[:], AF.Sqrt,
                        bias=qn9[:, qt_c : qt_c + 1],
                        scale=1.0 / (S - 1),
                    )
                    nc.sync.dma_start(
                        std_o[qt_c * 128 : (qt_c + 1) * 128, :], ostd_t[:]
                    )
                return emit

            m2_box = [None]
            ta = mk_tail_a(qt_, maccD, maccP, list(pend_pe))
            pend_pe = []

            def mk_a(ta_fn, box):
                def emit():
                    box[0] = ta_fn()
                return emit

            pend_tail_a = [mk_a(ta, m2_box)]
            pend_tail_b = [mk_tail_b(qt_, m2_box)]
        for fn in pend_tail_a:
            fn()
        for fn in pend_tail_b:
            fn()


def _prep_inputs(query_features, prototypes, weight_mu, weight_rho, bias_mu, bias_rho, eps_w, eps_b):
    f32, f16 = np.float32, np.float16
    sp_w = np.log1p(np.exp(weight_rho.astype(np.float64))).astype(f32)
    sp_b = np.log1p(np.exp(bias_rho.astype(np.float64))).astype(f32)
    W = (weight_mu[None] + eps_w * sp_w[None]).astype(f32)  # [S,D,D]
    B = (bias_mu[None] + eps_b * sp_b[None]).astype(f32)  # [S,D]
    b2 = (-2.0 * B).astype(f32)  # [S,D]

    y8 = prototypes.astype(f32).astype(E4)  # [P,D] fp8 prototypes
    y8f = y8.astype(f32)
    pn = (y8f.astype(np.float64) ** 2).sum(-1).astype(f32)  # [P]
    pn_hi = (pn * 0.5).astype(E4)
    pn_lo = (pn - 2.0 * pn_hi.astype(f32)).astype(f32).astype(E4)
    pn_seed = (2.0 * pn_hi.astype(f32) + pn_lo.astype(f32)).astype(f32)
    pn10_16 = (float(S) * pn_seed).astype(f16)[None, :]  # [1,P]

    W32 = np.ascontiguousarray(
        W.reshape(S, DT, 128, 256).transpose(0, 2, 1, 3).reshape(S, 128, DT * 256)
    )
    b2T = np.ascontiguousarray(
        b2.T.reshape(ET, 128, S).transpose(1, 0, 2).reshape(128, ET * S)
    )
    yT8 = np.ascontiguousarray(
        y8.T.reshape(ET, 128, P).transpose(1, 0, 2).reshape(128, ET * P)
    )
    y8dup = np.concatenate([yT8, yT8], axis=1)  # [128, 2*ET*P]
    yT16 = yT8.astype(f16)  # exact fp8 -> fp16
    seedw8 = np.concatenate(
        [np.full((1, 128), 2.0, E4), np.full((1, 128), 1.0, E4)], axis=1
    )
    seedy8 = np.concatenate([pn_hi[None, :], pn_lo[None, :]], axis=1)  # [1,2P]
    eye8dup = np.concatenate([np.eye(128, dtype=E4)] * 2, axis=1)
    sel16 = np.zeros((128, S * 16), f16)
    for s in range(S):
        sel16[:, s * 16 + s] = 1.0

    common = {
        "W32": W32,
        "b2T": b2T,
        "y8dup": np.ascontiguousarray(y8dup),
        "yT16": np.ascontiguousarray(yT16),
        "seedw8": seedw8,
        "seedy8": np.ascontiguousarray(seedy8),
        "pn10_16": pn10_16,
        "ones16": np.ones((1, 128), f16),
        "eye32": np.eye(128, dtype=f32),
        "eye8dup": np.ascontiguousarray(eye8dup),
        "eyeT10": np.eye(10, dtype=f32),
        "sel16": sel16,
    }
    qf = query_features.astype(f32)
    in_maps = []
    for c in range(NCORES):
        qs = qf[c * QLOC : (c + 1) * QLOC]  # [QLOC, D]
        qT32 = np.ascontiguousarray(
            qs.T.reshape(DT, 128, QLOC).transpose(1, 0, 2).reshape(128, DT * QLOC)
        )
        in_maps.append({"qT32": qT32, **common})
    return in_maps


def kernel(**inputs):
    global LAST_RESULTS
    n_samples = int(inputs.pop("n_samples", S))
    assert n_samples == S, f"kernel hardcodes S={S}, got {n_samples}"
    np_inputs = {
        k: np.asarray(v, dtype=np.float32)
        for k, v in inputs.items()
    }
    in_maps = _prep_inputs(**np_inputs)

    if "nc" not in _CACHE:
        _CACHE["nc"] = _build_bass()
    nc = _CACHE["nc"]

    trace = bool(int(os.environ.get("KERNEL_TRACE", "0")))
    res = bass_utils.run_bass_kernel_spmd(
        nc, in_maps, core_ids=list(range(NCORES)), trace=trace
    )
    LAST_RESULTS = res
    mean = np.concatenate([r["mean_o"] for r in res.results], axis=0)
    std = np.concatenate([r["std_o"] for r in res.results], axis=0)
    return mean, std


# revision 41
# speedup vs baseline: 1.9292x; 1.2371x over previous
"""Bayesian uncertainty distance kernel for TRN2 (8 NeuronCores, SPMD).

Math (per reference):
    W_s  = weight_mu + eps_w[s] * softplus(weight_rho)          [S,D,D]
    b_s  = bias_mu   + eps_b[s] * softplus(bias_rho)            [S,D]
    qt_s = query @ W_s + b_s                                    [S,Q,D]
    d2_s = ||qt_s||^2 - 2 qt_s.proto^T + ||proto||^2            [S,Q,P]
    mean = mean_s sqrt(d2_s);  std = std_s(sqrt(d2_s), ddof=1)

Sharding: data-parallel over Q (8192 -> 8 x 1024). Everything else replicated.

On-chip design (per core, Q=1024, P=2048, D=256, S=10), ~405us measured:
  - samples are DEFINED as x_s := fp16(-2*(query@W_s + b_s)) so that every
    moment is computed consistently from the same rounded values; first-order
    fp16 rounding error then cancels exactly in the variance (an inconsistent
    16-bit path measured 27% std error from catastrophic cancellation in
    E[d^2]-E[d]^2; this consistent one measures ~1.6e-3).
  - phase 1 per s: fp16 qt matmuls (W_s stationary, query^T moving) ->
    x_s = DVE tensor_scalar(psum*-2 + (-2 b_s)) -> fp16 [e,q] SBUF;
    x2 = ACT Square(x_s); qn rows = ones-stationary matmul of x2
    (scale 0.25 on the psum->sbuf copy), stored as fp16 rows with a
    companion all-ones row for the rank-2 update below.
  - xsum = sum_s x_s via identity-matmul PSUM accumulation (mixed-dtype
    DVE tensor_tensor measured 13x slow); qnsum row = DVE reduce of qn rows.
  - phase 2 per (qtile, s): PSUM d2 = rank-2([qn_s;1] x [1;pn]) +
    x_s-block @ proto^T (fp16, K=2x128, lhsT-major order to minimize
    LDWEIGHTS boundaries); dist = ACT Sqrt(psum), no bias needed;
    macc += dist (DVE fp32).
  - variance via sum-of-d2: ss = rank-2([qnsum;1] x [1;10*pn]) +
    xsum @ proto^T in fp32; u = ss - macc^2/10 (DVE); std = Sqrt(u/9).
  - mean = macc/10 on DVE (gpsimd tensor_scalar measured 29us/tile).

The host does only O(S*D^2) prep in numpy (softplus, W_s, transposes, pn).
"""

import os
import numpy as np

import concourse.bass as bass
import concourse.mybir as mybir
import concourse.tile as tile
from concourse import bacc, bass_utils

AF = mybir.ActivationFunctionType
ALU = mybir.AluOpType

# Note: walrus's --enable-ldw-opt stays false — fp32 matmuls emit
# InstLdweights that are "not compatible with LDW optimization".
F32 = mybir.dt.float32
F16 = mybir.dt.float16

NCORES = 8
D = 256
Q_FULL = 8192
P = 2048
S = 10
QLOC = Q_FULL // NCORES  # 1024
ET = D // 128  # 2 e-tiles
DT = D // 128  # 2 d-tiles
QT = QLOC // 128  # 8 q-tiles per core
PC = P // 512  # 4 p-chunks
QC = QLOC // 512  # 2 q-chunks

_CACHE = {}
LAST_RESULTS = None


def _build_bass():
    nc = bacc.Bacc(
        "TRN2",
        target_bir_lowering=False,
        debug=False,
        num_devices=NCORES,
    )
    ins = {}
    ins["qT16"] = nc.dram_tensor("qT16", [128, DT * QLOC], F16, kind="ExternalInput").ap()
    ins["W16"] = nc.dram_tensor("W16", [S, 128, DT * 256], F16, kind="ExternalInput").ap()
    ins["b2T"] = nc.dram_tensor("b2T", [128, ET * S], F32, kind="ExternalInput").ap()
    ins["yT16"] = nc.dram_tensor("yT16", [128, ET * P], F16, kind="ExternalInput").ap()
    ins["yext16"] = nc.dram_tensor("yext16", [2, P], F16, kind="ExternalInput").ap()
    ins["pn10_16"] = nc.dram_tensor("pn10_16", [1, P], F16, kind="ExternalInput").ap()
    ins["onesr16"] = nc.dram_tensor("onesr16", [1, 128], F16, kind="ExternalInput").ap()
    ins["eyeT1"] = nc.dram_tensor("eyeT1", [1, 1], F32, kind="ExternalInput").ap()
    ins["o16c"] = nc.dram_tensor("o16c", [128, 1], F16, kind="ExternalInput").ap()
    ins["eye16"] = nc.dram_tensor("eye16", [128, 128], F16, kind="ExternalInput").ap()
    mean_o = nc.dram_tensor("mean_o", [QLOC, P], F32, kind="ExternalOutput").ap()
    std_o = nc.dram_tensor("std_o", [QLOC, P], F32, kind="ExternalOutput").ap()

    with tile.TileContext(nc) as tc:
        _kernel_body(tc, ins, mean_o, std_o)
    nc.compile()
    return nc


def _kernel_body(tc, ins, mean_o, std_o):
    nc = tc.nc
    from contextlib import ExitStack

    ctx = ExitStack()
    with ctx:
        cpool = ctx.enter_context(tc.tile_pool(name="consts", bufs=1))
        wpool = ctx.enter_context(tc.tile_pool(name="wpool", bufs=2))
        xpool = ctx.enter_context(tc.tile_pool(name="xpool", bufs=S))
        x2pool = ctx.enter_context(tc.tile_pool(name="x2pool", bufs=2))
        xsumpool = ctx.enter_context(tc.tile_pool(name="xsumpool", bufs=1))
        qnpool = ctx.enter_context(tc.tile_pool(name="qnpool", bufs=1))
        distpool = ctx.enter_context(tc.tile_pool(name="distpool", bufs=3))
        maccpool = ctx.enter_context(tc.tile_pool(name="maccpool", bufs=2))
        finpool = ctx.enter_context(tc.tile_pool(name="finpool", bufs=2))
        outpool = ctx.enter_context(tc.tile_pool(name="outpool", bufs=3))
        pp = ctx.enter_context(tc.tile_pool(name="pp", bufs=4, space="PSUM"))

        # ---- constants into SBUF ----
        qT_t = cpool.tile([128, DT * QLOC], F16)
        nc.sync.dma_start(qT_t[:], ins["qT16"])
        b2_t = cpool.tile([128, ET * S], F32)
        nc.sync.dma_start(b2_t[:], ins["b2T"])
        yT16_t = cpool.tile([128, ET * P], F16)
        nc.sync.dma_start(yT16_t[:], ins["yT16"])
        yext16_t = cpool.tile([2, P], F16)
        nc.sync.dma_start(yext16_t[:], ins["yext16"])
        pn10_t = cpool.tile([1, P], F16)
        nc.sync.dma_start(pn10_t[:], ins["pn10_16"])
        onesr16_t = cpool.tile([1, 128], F16)
        nc.sync.dma_start(onesr16_t[:], ins["onesr16"])
        eyeT1_t = cpool.tile([1, 1], F32)
        nc.sync.dma_start(eyeT1_t[:], ins["eyeT1"])
        o16c_t = cpool.tile([128, 1], F16)
        nc.sync.dma_start(o16c_t[:], ins["o16c"])
        eye16_t = cpool.tile([128, 128], F16)
        nc.sync.dma_start(eye16_t[:], ins["eye16"])

        xsum_t = xsumpool.tile([128, ET * QLOC], F16)
        qn9 = qnpool.tile([128, QT], F32)  # qnsum/(S-1) bias columns for std
        # qn rows (fp16, max qn ~55k < 65504): row 0 holds qn for all (s,q),
        # row 1 is ones; [2,128] slices feed the rank-2 (qn+pn) matmul.
        qrow16_t = qnpool.tile([2, S * QLOC], F16)
        nc.vector.memset(qrow16_t[0:2, :], 1.0)
        # ss-side rank-2 operand: row 0 = qnsum (fp32), row 1 = ones
        qsrow32_t = qnpool.tile([2, QLOC], F32)
        nc.vector.memset(qsrow32_t[0:2, :], 1.0)

        x_tiles = []
        # ---------- phase 1: per-sample transformed queries ----------
        for s in range(S):
            w_t = wpool.tile([128, DT * 256], F16, tag="w")
            nc.sync.dma_start(w_t[:], ins["W16"][s])
            x_t = xpool.tile([128, ET * QLOC], F16, tag="x", name=f"x{s}")
            x_tiles.append(x_t)
            x2s = []
            for et in range(ET):
                for qc in range(QC):
                    qp = pp.tile([128, 512], F32, tag="ps", name=f"qp{s}_{et}_{qc}")
                    for dt_ in range(DT):
                        nc.tensor.matmul(
                            qp[:],
                            lhsT=w_t[:, dt_ * 256 + et * 128 : dt_ * 256 + et * 128 + 128],
                            rhs=qT_t[:, dt_ * QLOC + qc * 512 : dt_ * QLOC + qc * 512 + 512],
                            start=(dt_ == 0),
                            stop=(dt_ == DT - 1),
                        )
                    # x = fp16(-2*qt - 2*b) on DVE: (psum * -2) + b2col
                    # (keeps phase-1 ACT light so the PE stream stays dense)
                    nc.vector.tensor_scalar(
                        x_t[:, et * QLOC + qc * 512 : et * QLOC + qc * 512 + 512],
                        qp[:],
                        -2.0,
                        b2_t[:, et * S + s : et * S + s + 1],
                        ALU.mult,
                        ALU.add,
                    )
                x2_t = x2pool.tile([128, QLOC], F16, tag=f"x2_{et}", name=f"x2_{s}_{et}")
                x2s.append(x2_t)
                # x2 = x^2 = 4*qt^2 on ACT (phase 1 is DVE-bound; the 0.25
                # compensation is folded into the qn psum->sbuf copy scale)
                nc.scalar.square(x2_t[:], x_t[:, et * QLOC : (et + 1) * QLOC])
            # qn rows: ones-stationary matmuls (shared lhsT, no LDW tax);
            # 0.25 compensates x2 = (2*qt)^2
            for qc in range(QC):
                qr_p = pp.tile([1, 512], F32, tag="ps", name=f"qr{s}_{qc}")
                for et in range(ET):
                    nc.tensor.matmul(
                        qr_p[:],
                        lhsT=o16c_t[:],
                        rhs=x2s[et][:, qc * 512 : (qc + 1) * 512],
                        start=(et == 0),
                        stop=(et == ET - 1),
                    )
                nc.scalar.mul(
                    qrow16_t[0:1, s * QLOC + qc * 512 : s * QLOC + qc * 512 + 512],
                    qr_p[:],
                    0.25,
                )

        # xsum = sum_s x_s via identity-matmul PSUM accumulation (a mixed
        # fp16+fp32 DVE tensor_tensor measured 13x slower than fp32+fp32,
        # so the PE does the accumulation instead)
        for et in range(ET):
            for qc in range(QC):
                xsp = pp.tile([128, 512], F32, tag="ps", name=f"xsp{et}_{qc}")
                for s in range(S):
                    nc.tensor.matmul(
                        xsp[:],
                        lhsT=eye16_t[:],
                        rhs=x_tiles[s][
                            :, et * QLOC + qc * 512 : et * QLOC + qc * 512 + 512
                        ],
                        start=(s == 0),
                        stop=(s == S - 1),
                    )
                nc.scalar.activation(
                    xsum_t[:, et * QLOC + qc * 512 : et * QLOC + qc * 512 + 512],
                    xsp[:],
                    AF.Copy,
                )

        # qnsum row (fp32) = sum_s of the fp16 qn rows, consistent with the
        # per-sample values the rank-2 matmuls use
        nc.vector.tensor_reduce(
            qsrow32_t[0:1, :],
            qrow16_t[0:1, :].rearrange("p (s q) -> p q s", s=S),
            axis=mybir.AxisListType.X,
            op=ALU.add,
        )
        # qnsum/(S-1) as per-partition bias columns (PE transpose of the
        # qnsum row) so the ss matmuls can drop the fp32 rank-2 entirely
        for qt_ in range(QT):
            qsp = pp.tile([128, 1], F32, tag="ps", name=f"qsp{qt_}")
            nc.tensor.matmul(
                qsp[:],
                lhsT=qsrow32_t[0:1, qt_ * 128 : qt_ * 128 + 128],
                rhs=eyeT1_t[:],
                is_transpose=True,
            )
            nc.scalar.mul(qn9[:, qt_ : qt_ + 1], qsp[:], 1.0 / (S - 1))

        # ---------- phase 2: distances, moments, outputs ----------
        PH = 1024  # psum tile width (2 banks); 4 bufs deepen the PE pipeline
        NH = P // PH
        for qt_ in range(QT):
            macc_t = maccpool.tile([128, P], F32, tag="macc", name=f"macc{qt_}")
            for s in range(S):
                dist_t = None
                if s > 0:
                    dist_t = distpool.tile([128, P], F32, tag="dist", name=f"d{qt_}_{s}")
                cps = [
                    pp.tile([128, PH], F32, tag="ps", name=f"cp{qt_}_{s}_{h}")
                    for h in range(NH)
                ]
                # lhsT-major ordering: each stationary operand covers all
                # PSUM halves before switching (leader-MM LDW tax once per
                # lhsT instead of once per half)
                lhsT_r2 = qrow16_t[:, s * QLOC + qt_ * 128 : s * QLOC + qt_ * 128 + 128]
                for h in range(NH):
                    for pc in range(PH // 512):
                        o = h * PH + pc * 512
                        nc.tensor.matmul(
                            cps[h][:, pc * 512 : (pc + 1) * 512],
                            lhsT=lhsT_r2,
                            rhs=yext16_t[:, o : o + 512],
                            start=True,
                            stop=False,
                            skip_group_check=True,
                        )
                for et in range(ET):
                    lhs = x_tiles[s][
                        :, et * QLOC + qt_ * 128 : et * QLOC + qt_ * 128 + 128
                    ]
                    for h in range(NH):
                        for pc in range(PH // 512):
                            o = h * PH + pc * 512
                            nc.tensor.matmul(
                                cps[h][:, pc * 512 : (pc + 1) * 512],
                                lhsT=lhs,
                                rhs=yT16_t[:, et * P + o : et * P + o + 512],
                                start=False,
                                stop=(et == ET - 1),
                                skip_group_check=True,
                            )
                dst = macc_t if s == 0 else dist_t
                for h in range(NH):
                    nc.scalar.activation(
                        dst[:, h * PH : (h + 1) * PH], cps[h][:], AF.Sqrt
                    )
                if s > 0:
                    nc.vector.tensor_add(macc_t[:], macc_t[:], dist_t[:])

            # sum_s d2 = qnsum + 10*pn + xsum.proto^T (fp32, rank-2 + cross)
            # m2 = macc^2; u = ssp - m2/10  (all on DVE, ACT stays on sqrt)
            m2_t = finpool.tile([128, P], F32, tag="fin", name=f"m2{qt_}")
            nc.vector.tensor_mul(m2_t[:], macc_t[:], macc_t[:])
            u_t = finpool.tile([128, P], F32, tag="fin", name=f"u{qt_}")
            ssps = [
                pp.tile([128, PH], F32, tag="ps", name=f"ssp{qt_}_{h}")
                for h in range(NH)
            ]
            # rank-1 10*pn seed + fp16 cross (exact accumulation); qnsum
            # joins at the final Sqrt as a per-partition bias
            for h in range(NH):
                for pc in range(PH // 512):
                    o = h * PH + pc * 512
                    nc.tensor.matmul(
                        ssps[h][:, pc * 512 : (pc + 1) * 512],
                        lhsT=onesr16_t[:],
                        rhs=pn10_t[:, o : o + 512],
                        start=True,
                        stop=False,
                        skip_group_check=True,
                    )
            for et in range(ET):
                lhs = xsum_t[:, et * QLOC + qt_ * 128 : et * QLOC + qt_ * 128 + 128]
                for h in range(NH):
                    for pc in range(PH // 512):
                        o = h * PH + pc * 512
                        nc.tensor.matmul(
                            ssps[h][:, pc * 512 : (pc + 1) * 512],
                            lhsT=lhs,
                            rhs=yT16_t[:, et * P + o : et * P + o + 512],
                            start=False,
                            stop=(et == ET - 1),
                            skip_group_check=True,
                        )
            for h in range(NH):
                nc.vector.scalar_tensor_tensor(
                    u_t[:, h * PH : (h + 1) * PH],
                    m2_t[:, h * PH : (h + 1) * PH],
                    -1.0 / S,
                    ssps[h][:],
                    ALU.mult,
                    ALU.add,
                )
            ostd_t = outpool.tile([128, P], F32, tag="out", name=f"os{qt_}")
            nc.scalar.activation(
                ostd_t[:], u_t[:], AF.Sqrt,
                bias=qn9[:, qt_ : qt_ + 1],
                scale=1.0 / (S - 1),
            )
            omean_t = outpool.tile([128, P], F32, tag="out", name=f"om{qt_}")
            nc.vector.tensor_scalar_mul(omean_t[:], macc_t[:], 1.0 / S)
            nc.sync.dma_start(std_o[qt_ * 128 : (qt_ + 1) * 128, :], ostd_t[:])
            nc.sync.dma_start(mean_o[qt_ * 128 : (qt_ + 1) * 128, :], omean_t[:])


def _prep_inputs(query_features, prototypes, weight_mu, weight_rho, bias_mu, bias_rho, eps_w, eps_b):
    f32, f16 = np.float32, np.float16
    sp_w = np.log1p(np.exp(weight_rho.astype(np.float64))).astype(f32)
    sp_b = np.log1p(np.exp(bias_rho.astype(np.float64))).astype(f32)
    W = (weight_mu[None] + eps_w * sp_w[None]).astype(f32)  # [S,D,D]
    B = (bias_mu[None] + eps_b * sp_b[None]).astype(f32)  # [S,D]
    Wh = W.astype(f16)
    qfh = query_features.astype(f16)  # [Q,D]
    yh = prototypes.astype(f16)  # [P,D]
    pn = (yh.astype(f32) ** 2).sum(-1, dtype=f32)  # [P]
    pn16 = pn.astype(f16)
    pn10 = (float(S) * pn16.astype(f32)).astype(f32)
    b2 = (-2.0 * B).astype(f32)  # [S,D]

    W16 = np.ascontiguousarray(
        Wh.reshape(S, DT, 128, 256).transpose(0, 2, 1, 3).reshape(S, 128, DT * 256)
    )
    b2T = np.ascontiguousarray(
        b2.T.reshape(ET, 128, S).transpose(1, 0, 2).reshape(128, ET * S)
    )
    yT16 = np.ascontiguousarray(
        yh.T.reshape(ET, 128, P).transpose(1, 0, 2).reshape(128, ET * P)
    )
    yext16 = np.stack([np.ones(P, f16), pn16]).astype(f16)  # [2,P]
    pn10_16 = pn10.astype(f16)[None, :]  # [1,P]
    common = {
        "W16": W16,
        "b2T": b2T,
        "yT16": yT16,
        "yext16": yext16,
        "pn10_16": pn10_16,
        "onesr16": np.ones((1, 128), f16),
        "eyeT1": np.eye(1, dtype=f32),
        "o16c": np.ones((128, 1), f16),
        "eye16": np.eye(128, dtype=f16),
    }
    in_maps = []
    for c in range(NCORES):
        qs = qfh[c * QLOC : (c + 1) * QLOC]  # [QLOC, D]
        qT16 = np.ascontiguousarray(
            qs.T.reshape(DT, 128, QLOC).transpose(1, 0, 2).reshape(128, DT * QLOC)
        )
        in_maps.append({"qT16": qT16, **common})
    return in_maps


def kernel(**inputs):
    global LAST_RESULTS
    n_samples = int(inputs.pop("n_samples", S))
    assert n_samples == S, f"kernel hardcodes S={S}, got {n_samples}"
    np_inputs = {
        k: np.asarray(v, dtype=np.float32)
        for k, v in inputs.items()
    }
    in_maps = _prep_inputs(**np_inputs)

    if "nc" not in _CACHE:
        _CACHE["nc"] = _build_bass()
    nc = _CACHE["nc"]

    trace = bool(int(os.environ.get("KERNEL_TRACE", "0")))
    res = bass_utils.run_bass_kernel_spmd(
        nc, in_maps, core_ids=list(range(NCORES)), trace=trace
    )
    LAST_RESULTS = res
    mean = np.concatenate([r["mean_o"] for r in res.results], axis=0)
    std = np.concatenate([r["std_o"] for r in res.results], axis=0)
    return mean, std



# revision 52
# speedup vs baseline: 1.9449x; 1.0081x over previous
"""Bayesian uncertainty distance kernel for TRN2 (8 NeuronCores, SPMD).

Math (per reference):
    W_s  = weight_mu + eps_w[s] * softplus(weight_rho)          [S,D,D]
    b_s  = bias_mu   + eps_b[s] * softplus(bias_rho)            [S,D]
    qt_s = query @ W_s + b_s                                    [S,Q,D]
    d2_s = ||qt_s||^2 - 2 qt_s.proto^T + ||proto||^2            [S,Q,P]
    mean = mean_s sqrt(d2_s);  std = std_s(sqrt(d2_s), ddof=1)

Sharding: data-parallel over Q (8192 -> 8 x 1024). Everything else replicated.

On-chip design (per core, Q=1024, P=2048, D=256, S=10), ~405us measured:
  - samples are DEFINED as x_s := fp16(-2*(query@W_s + b_s)) so that every
    moment is computed consistently from the same rounded values; first-order
    fp16 rounding error then cancels exactly in the variance (an inconsistent
    16-bit path measured 27% std error from catastrophic cancellation in
    E[d^2]-E[d]^2; this consistent one measures ~1.6e-3).
  - phase 1 per s: fp16 qt matmuls (W_s stationary, query^T moving) ->
    x_s = DVE tensor_scalar(psum*-2 + (-2 b_s)) -> fp16 [e,q] SBUF;
    x2 = ACT Square(x_s); qn rows = ones-stationary matmul of x2
    (scale 0.25 on the psum->sbuf copy), stored as fp16 rows with a
    companion all-ones row for the rank-2 update below.
  - xsum = sum_s x_s via identity-matmul PSUM accumulation (mixed-dtype
    DVE tensor_tensor measured 13x slow); qnsum row = DVE reduce of qn rows.
  - phase 2 per (qtile, s): PSUM d2 = rank-2([qn_s;1] x [1;pn]) +
    x_s-block @ proto^T (fp16, K=2x128, lhsT-major order to minimize
    LDWEIGHTS boundaries); dist = ACT Sqrt(psum), no bias needed;
    macc += dist (DVE fp32).
  - variance via sum-of-d2: ss = rank-2([qnsum;1] x [1;10*pn]) +
    xsum @ proto^T in fp32; u = ss - macc^2/10 (DVE); std = Sqrt(u/9).
  - mean = macc/10 on DVE (gpsimd tensor_scalar measured 29us/tile).

The host does only O(S*D^2) prep in numpy (softplus, W_s, transposes, pn).
"""

import os
import numpy as np

import concourse.bass as bass
import concourse.mybir as mybir
import concourse.tile as tile
from concourse import bacc, bass_utils

AF = mybir.ActivationFunctionType
ALU = mybir.AluOpType

# Note: walrus's --enable-ldw-opt stays false — fp32 matmuls emit
# InstLdweights that are "not compatible with LDW optimization".
F32 = mybir.dt.float32
F16 = mybir.dt.float16

NCORES = 8
D = 256
Q_FULL = 8192
P = 2048
S = 10
QLOC = Q_FULL // NCORES  # 1024
ET = D // 128  # 2 e-tiles
DT = D // 128  # 2 d-tiles
QT = QLOC // 128  # 8 q-tiles per core
PC = P // 512  # 4 p-chunks
QC = QLOC // 512  # 2 q-chunks

_CACHE = {}
LAST_RESULTS = None


def _build_bass():
    nc = bacc.Bacc(
        "TRN2",
        target_bir_lowering=False,
        debug=False,
        num_devices=NCORES,
    )
    ins = {}
    ins["qT16"] = nc.dram_tensor("qT16", [128, DT * QLOC], F16, kind="ExternalInput").ap()
    ins["W16"] = nc.dram_tensor("W16", [S, 128, DT * 256], F16, kind="ExternalInput").ap()
    ins["b2T"] = nc.dram_tensor("b2T", [128, ET * S], F32, kind="ExternalInput").ap()
    ins["yT16"] = nc.dram_tensor("yT16", [128, ET * P], F16, kind="ExternalInput").ap()
    ins["yext16"] = nc.dram_tensor("yext16", [2, P], F16, kind="ExternalInput").ap()
    ins["pn10_16"] = nc.dram_tensor("pn10_16", [1, P], F16, kind="ExternalInput").ap()
    ins["onesr16"] = nc.dram_tensor("onesr16", [1, 128], F16, kind="ExternalInput").ap()
    ins["eyeT1"] = nc.dram_tensor("eyeT1", [1, 1], F32, kind="ExternalInput").ap()
    ins["o16c"] = nc.dram_tensor("o16c", [128, 1], F16, kind="ExternalInput").ap()
    ins["eye16"] = nc.dram_tensor("eye16", [128, 128], F16, kind="ExternalInput").ap()
    mean_o = nc.dram_tensor("mean_o", [QLOC, P], F32, kind="ExternalOutput").ap()
    std_o = nc.dram_tensor("std_o", [QLOC, P], F32, kind="ExternalOutput").ap()

    with tile.TileContext(nc) as tc:
        _kernel_body(tc, ins, mean_o, std_o)
    nc.compile()
    return nc


def _kernel_body(tc, ins, mean_o, std_o):
    nc = tc.nc
    from contextlib import ExitStack

    ctx = ExitStack()
    with ctx:
        cpool = ctx.enter_context(tc.tile_pool(name="consts", bufs=1))
        wpool = ctx.enter_context(tc.tile_pool(name="wpool", bufs=2))
        xpool = ctx.enter_context(tc.tile_pool(name="xpool", bufs=S))
        x2pool = ctx.enter_context(tc.tile_pool(name="x2pool", bufs=2))
        xsumpool = ctx.enter_context(tc.tile_pool(name="xsumpool", bufs=1))
        qnpool = ctx.enter_context(tc.tile_pool(name="qnpool", bufs=1))
        distpool = ctx.enter_context(tc.tile_pool(name="distpool", bufs=3))
        maccpool = ctx.enter_context(tc.tile_pool(name="maccpool", bufs=2))
        finpool = ctx.enter_context(tc.tile_pool(name="finpool", bufs=2))
        outpool = ctx.enter_context(tc.tile_pool(name="outpool", bufs=3))
        pp = ctx.enter_context(tc.tile_pool(name="pp", bufs=4, space="PSUM"))

        # ---- constants into SBUF ----
        qT_t = cpool.tile([128, DT * QLOC], F16)
        nc.sync.dma_start(qT_t[:], ins["qT16"])
        b2_t = cpool.tile([128, ET * S], F32)
        nc.sync.dma_start(b2_t[:], ins["b2T"])
        yT16_t = cpool.tile([128, ET * P], F16)
        nc.sync.dma_start(yT16_t[:], ins["yT16"])
        yext16_t = cpool.tile([2, P], F16)
        nc.sync.dma_start(yext16_t[:], ins["yext16"])
        pn10_t = cpool.tile([1, P], F16)
        nc.sync.dma_start(pn10_t[:], ins["pn10_16"])
        onesr16_t = cpool.tile([1, 128], F16)
        nc.sync.dma_start(onesr16_t[:], ins["onesr16"])
        eyeT1_t = cpool.tile([1, 1], F32)
        nc.sync.dma_start(eyeT1_t[:], ins["eyeT1"])
        o16c_t = cpool.tile([128, 1], F16)
        nc.sync.dma_start(o16c_t[:], ins["o16c"])
        eye16_t = cpool.tile([128, 128], F16)
        nc.sync.dma_start(eye16_t[:], ins["eye16"])

        xsum_t = xsumpool.tile([128, ET * QLOC], F16)
        qn9 = qnpool.tile([128, QT], F32)  # qnsum/(S-1) bias columns for std
        # qn rows (fp16, max qn ~55k < 65504): row 0 holds qn for all (s,q),
        # row 1 is ones; [2,128] slices feed the rank-2 (qn+pn) matmul.
        qrow16_t = qnpool.tile([2, S * QLOC], F16)
        nc.vector.memset(qrow16_t[0:2, :], 1.0)
        # ss-side rank-2 operand: row 0 = qnsum (fp32), row 1 = ones
        qsrow32_t = qnpool.tile([2, QLOC], F32)
        nc.vector.memset(qsrow32_t[0:2, :], 1.0)

        x_tiles = []
        # ---------- phase 1: per-sample transformed queries ----------
        for s in range(S):
            w_t = wpool.tile([128, DT * 256], F16, tag="w")
            nc.sync.dma_start(w_t[:], ins["W16"][s])
            x_t = xpool.tile([128, ET * QLOC], F16, tag="x", name=f"x{s}")
            x_tiles.append(x_t)
            x2s = []
            for et in range(ET):
                for qc in range(QC):
                    qp = pp.tile([128, 512], F32, tag="ps", name=f"qp{s}_{et}_{qc}")
                    for dt_ in range(DT):
                        nc.tensor.matmul(
                            qp[:],
                            lhsT=w_t[:, dt_ * 256 + et * 128 : dt_ * 256 + et * 128 + 128],
                            rhs=qT_t[:, dt_ * QLOC + qc * 512 : dt_ * QLOC + qc * 512 + 512],
                            start=(dt_ == 0),
                            stop=(dt_ == DT - 1),
                        )
                    # x = fp16(-2*qt - 2*b) on DVE: (psum * -2) + b2col
                    # (keeps phase-1 ACT light so the PE stream stays dense)
                    nc.vector.tensor_scalar(
                        x_t[:, et * QLOC + qc * 512 : et * QLOC + qc * 512 + 512],
                        qp[:],
                        -2.0,
                        b2_t[:, et * S + s : et * S + s + 1],
                        ALU.mult,
                        ALU.add,
                    )
                x2_t = x2pool.tile([128, QLOC], F16, tag=f"x2_{et}", name=f"x2_{s}_{et}")
                x2s.append(x2_t)
                # x2 = x^2 = 4*qt^2 on ACT (phase 1 is DVE-bound; the 0.25
                # compensation is folded into the qn psum->sbuf copy scale)
                nc.scalar.square(x2_t[:], x_t[:, et * QLOC : (et + 1) * QLOC])
            # qn rows: ones-stationary matmuls (shared lhsT, no LDW tax);
            # 0.25 compensates x2 = (2*qt)^2
            for qc in range(QC):
                qr_p = pp.tile([1, 512], F32, tag="ps", name=f"qr{s}_{qc}")
                for et in range(ET):
                    nc.tensor.matmul(
                        qr_p[:],
                        lhsT=o16c_t[:],
                        rhs=x2s[et][:, qc * 512 : (qc + 1) * 512],
                        start=(et == 0),
                        stop=(et == ET - 1),
                    )
                nc.scalar.mul(
                    qrow16_t[0:1, s * QLOC + qc * 512 : s * QLOC + qc * 512 + 512],
                    qr_p[:],
                    0.25,
                )

        # xsum = sum_s x_s via identity-matmul PSUM accumulation (a mixed
        # fp16+fp32 DVE tensor_tensor measured 13x slower than fp32+fp32,
        # so the PE does the accumulation instead)
        for et in range(ET):
            for qc in range(QC):
                xsp = pp.tile([128, 512], F32, tag="ps", name=f"xsp{et}_{qc}")
                for s in range(S):
                    nc.tensor.matmul(
                        xsp[:],
                        lhsT=eye16_t[:],
                        rhs=x_tiles[s][
                            :, et * QLOC + qc * 512 : et * QLOC + qc * 512 + 512
                        ],
                        start=(s == 0),
                        stop=(s == S - 1),
                    )
                nc.scalar.activation(
                    xsum_t[:, et * QLOC + qc * 512 : et * QLOC + qc * 512 + 512],
                    xsp[:],
                    AF.Copy,
                )

        # qnsum row (fp32) = sum_s of the fp16 qn rows, consistent with the
        # per-sample values the rank-2 matmuls use
        nc.vector.tensor_reduce(
            qsrow32_t[0:1, :],
            qrow16_t[0:1, :].rearrange("p (s q) -> p q s", s=S),
            axis=mybir.AxisListType.X,
            op=ALU.add,
        )
        # qnsum/(S-1) as per-partition bias columns (PE transpose of the
        # qnsum row) so the ss matmuls can drop the fp32 rank-2 entirely
        for qt_ in range(QT):
            qsp = pp.tile([128, 1], F32, tag="ps", name=f"qsp{qt_}")
            nc.tensor.matmul(
                qsp[:],
                lhsT=qsrow32_t[0:1, qt_ * 128 : qt_ * 128 + 128],
                rhs=eyeT1_t[:],
                is_transpose=True,
            )
            nc.scalar.mul(qn9[:, qt_ : qt_ + 1], qsp[:], 1.0 / (S - 1))

        # ---------- phase 2: distances, moments, outputs ----------
        PH = 1024  # psum tile width (2 banks); 4 bufs deepen the PE pipeline
        NH = P // PH
        for qt_ in range(QT):
            macc_t = maccpool.tile([128, P], F32, tag="macc", name=f"macc{qt_}")
            for s in range(S):
                dist_t = None
                if s > 0:
                    dist_t = distpool.tile([128, P], F32, tag="dist", name=f"d{qt_}_{s}")
                cps = [
                    pp.tile([128, PH], F32, tag="ps", name=f"cp{qt_}_{s}_{h}")
                    for h in range(NH)
                ]
                # lhsT-major ordering: each stationary operand covers all
                # PSUM halves before switching (leader-MM LDW tax once per
                # lhsT instead of once per half)
                lhsT_r2 = qrow16_t[:, s * QLOC + qt_ * 128 : s * QLOC + qt_ * 128 + 128]
                for h in range(NH):
                    for pc in range(PH // 512):
                        o = h * PH + pc * 512
                        nc.tensor.matmul(
                            cps[h][:, pc * 512 : (pc + 1) * 512],
                            lhsT=lhsT_r2,
                            rhs=yext16_t[:, o : o + 512],
                            start=True,
                            stop=False,
                            skip_group_check=True,
                        )
                for et in range(ET):
                    lhs = x_tiles[s][
                        :, et * QLOC + qt_ * 128 : et * QLOC + qt_ * 128 + 128
                    ]
                    for h in range(NH):
                        for pc in range(PH // 512):
                            o = h * PH + pc * 512
                            nc.tensor.matmul(
                                cps[h][:, pc * 512 : (pc + 1) * 512],
                                lhsT=lhs,
                                rhs=yT16_t[:, et * P + o : et * P + o + 512],
                                start=False,
                                stop=(et == ET - 1),
                                skip_group_check=True,
                            )
                dst = macc_t if s == 0 else dist_t
                for h in range(NH):
                    nc.scalar.activation(
                        dst[:, h * PH : (h + 1) * PH], cps[h][:], AF.Sqrt
                    )
                if s > 0:
                    nc.vector.tensor_add(macc_t[:], macc_t[:], dist_t[:])

            # sum_s d2 = qnsum + 10*pn + xsum.proto^T (fp32, rank-2 + cross)
            # m2 = macc^2; u = ssp - m2/10  (all on DVE, ACT stays on sqrt)
            m2_t = finpool.tile([128, P], F32, tag="fin", name=f"m2{qt_}")
            nc.vector.tensor_mul(m2_t[:], macc_t[:], macc_t[:])
            u_t = finpool.tile([128, P], F32, tag="fin", name=f"u{qt_}")
            ssps = [
                pp.tile([128, PH], F32, tag="ps", name=f"ssp{qt_}_{h}")
                for h in range(NH)
            ]
            # rank-1 10*pn seed + fp16 cross (exact accumulation); qnsum
            # joins at the final Sqrt as a per-partition bias
            for h in range(NH):
                for pc in range(PH // 512):
                    o = h * PH + pc * 512
                    nc.tensor.matmul(
                        ssps[h][:, pc * 512 : (pc + 1) * 512],
                        lhsT=onesr16_t[:],
                        rhs=pn10_t[:, o : o + 512],
                        start=True,
                        stop=False,
                        skip_group_check=True,
                    )
            for et in range(ET):
                lhs = xsum_t[:, et * QLOC + qt_ * 128 : et * QLOC + qt_ * 128 + 128]
                for h in range(NH):
                    for pc in range(PH // 512):
                        o = h * PH + pc * 512
                        nc.tensor.matmul(
                            ssps[h][:, pc * 512 : (pc + 1) * 512],
                            lhsT=lhs,
                            rhs=yT16_t[:, et * P + o : et * P + o + 512],
                            start=False,
                            stop=(et == ET - 1),
                            skip_group_check=True,
                        )
            for h in range(NH):
                nc.vector.scalar_tensor_tensor(
                    u_t[:, h * PH : (h + 1) * PH],
                    m2_t[:, h * PH : (h + 1) * PH],
                    -1.0 / S,
                    ssps[h][:],
                    ALU.mult,
                    ALU.add,
                )
            ostd_t = outpool.tile([128, P], F32, tag="out", name=f"os{qt_}")
            nc.scalar.activation(
                ostd_t[:], u_t[:], AF.Sqrt,
                bias=qn9[:, qt_ : qt_ + 1],
                scale=1.0 / (S - 1),
            )
            omean_t = outpool.tile([128, P], F32, tag="out", name=f"om{qt_}")
            nc.vector.tensor_scalar_mul(omean_t[:], macc_t[:], 1.0 / S)
            nc.sync.dma_start(std_o[qt_ * 128 : (qt_ + 1) * 128, :], ostd_t[:])
            nc.sync.dma_start(mean_o[qt_ * 128 : (qt_ + 1) * 128, :], omean_t[:])


def _prep_inputs(query_features, prototypes, weight_mu, weight_rho, bias_mu, bias_rho, eps_w, eps_b):
    f32, f16 = np.float32, np.float16
    sp_w = np.log1p(np.exp(weight_rho.astype(np.float64))).astype(f32)
    sp_b = np.log1p(np.exp(bias_rho.astype(np.float64))).astype(f32)
    W = (weight_mu[None] + eps_w * sp_w[None]).astype(f32)  # [S,D,D]
    B = (bias_mu[None] + eps_b * sp_b[None]).astype(f32)  # [S,D]
    Wh = W.astype(f16)
    qfh = query_features.astype(f16)  # [Q,D]
    yh = prototypes.astype(f16)  # [P,D]
    pn = (yh.astype(f32) ** 2).sum(-1, dtype=f32)  # [P]
    pn16 = pn.astype(f16)
    pn10 = (float(S) * pn16.astype(f32)).astype(f32)
    b2 = (-2.0 * B).astype(f32)  # [S,D]

    W16 = np.ascontiguousarray(
        Wh.reshape(S, DT, 128, 256).transpose(0, 2, 1, 3).reshape(S, 128, DT * 256)
    )
    b2T = np.ascontiguousarray(
        b2.T.reshape(ET, 128, S).transpose(1, 0, 2).reshape(128, ET * S)
    )
    yT16 = np.ascontiguousarray(
        yh.T.reshape(ET, 128, P).transpose(1, 0, 2).reshape(128, ET * P)
    )
    yext16 = np.stack([np.ones(P, f16), pn16]).astype(f16)  # [2,P]
    pn10_16 = pn10.astype(f16)[None, :]  # [1,P]
    common = {
        "W16": W16,
        "b2T": b2T,
        "yT16": yT16,
        "yext16": yext16,
        "pn10_16": pn10_16,
        "onesr16": np.ones((1, 128), f16),
        "eyeT1": np.eye(1, dtype=f32),
        "o16c": np.ones((128, 1), f16),
        "eye16": np.eye(128, dtype=f16),
    }
    in_maps = []
    for c in range(NCORES):
        qs = qfh[c * QLOC : (c + 1) * QLOC]  # [QLOC, D]
        qT16 = np.ascontiguousarray(
            qs.T.reshape(DT, 128, QLOC).transpose(1, 0, 2).reshape(128, DT * QLOC)
        )
        in_maps.append({"qT16": qT16, **common})
    return in_maps


def kernel(**inputs):
    global LAST_RESULTS
    n_samples = int(inputs.pop("n_samples", S))
    assert n_samples == S, f"kernel hardcodes S={S}, got {n_samples}"
    np_inputs = {
        k: np.asarray(v, dtype=np.float32)
        for k, v in inputs.items()
    }
    in_maps = _prep_inputs(**np_inputs)

    if "nc" not in _CACHE:
        _CACHE["nc"] = _build_bass()
    nc = _CACHE["nc"]

    trace = bool(int(os.environ.get("KERNEL_TRACE", "0")))
    res = bass_utils.run_bass_kernel_spmd(
        nc, in_maps, core_ids=list(range(NCORES)), trace=trace
    )
    LAST_RESULTS = res
    mean = np.concatenate([r["mean_o"] for r in res.results], axis=0)
    std = np.concatenate([r["std_o"] for r in res.results], axis=0)
    return mean, std

